# revision 1
# baseline (speedup 1.0000x reference)
"""Binary-approximate sparse attention on 8 Trainium2 NeuronCores.

Reference semantics (per batch b, head h, query q):
  s      = sign(q) . sign(k)            -- integer scores in [-64, 64], even
  top-k  = 102 largest s, ties broken toward LOWER key index (jax.lax.top_k)
  out    = softmax over the precise scores (q.k/8) of the selected keys @ v

Device algorithm (per (b,h) pair; 24 pairs sharded 3-per-core):
  - approx scores via f16 matmuls (sign bits exact in f16)
  - per-query threshold score level found by 7-step interval bisection over
    the odd-integer lattice (exact; power-of-2 intervals so midpoints stay
    integral -- no floor needed)
  - tie cutoff index c_q = position of the r-th tie via a prefix-sum scan
    along the key axis + one fused count
  - selection mask applied WITHOUT any gather: augmented unique score
    v = s + (1023-k)/1024 compared against the augmented threshold
    tau = t + (1023-c_q)/1024 (all arithmetic exact in fp32/PSUM)
  - masked softmax + p@V computed in [key-partition, query-free] layout so
    the softmax denominator and output reduce over keys via PE matmuls
    (no large transposes anywhere)
"""

import os
from contextlib import ExitStack

KSTAGE = int(os.environ.get("KSTAGE", "9"))  # debug: truncate program

import numpy as np

import concourse.bacc as bacc
import concourse.bass as bass
import concourse.mybir as mybir
import concourse.tile as tile
from concourse.bass_utils import run_bass_kernel_spmd

B, H, S, D = 2, 12, 1024, 64
NCORES = 8
PAIRS = (B * H) // NCORES          # (b,h) pairs per core
KP = 102                           # top-k
QT = S // 128                      # 128-row tiles per axis
NH = S // 512                      # 512-col halves

F32 = mybir.dt.float32
F32R = mybir.dt.float32r
F16 = mybir.dt.float16
AF = mybir.ActivationFunctionType
OP = mybir.AluOpType


def _consts():
    ident32 = np.eye(128, dtype=np.float32)
    ident16 = np.eye(128, dtype=np.float16)
    # w[k] = (S-1-k)/S : augmentation giving unique scores, lower index wins
    wrow = (((S - 1) - np.arange(S, dtype=np.float32)) / S).astype(np.float16)[None, :]
    onesrow = np.ones((1, 512), dtype=np.float16)
    ones1x128 = np.ones((1, 128), dtype=np.float16)
    return ident32, ident16, wrow, onesrow, ones1x128


def build_program():
    nc = bacc.Bacc("TRN2", target_bir_lowering=False, debug=False,
                   num_devices=NCORES)

    qd = nc.dram_tensor("q_in", (PAIRS, S, D), F32, kind="ExternalInput").ap()
    kd = nc.dram_tensor("k_in", (PAIRS, S, D), F32, kind="ExternalInput").ap()
    vd = nc.dram_tensor("v_in", (PAIRS, S, D), F32, kind="ExternalInput").ap()
    identd = nc.dram_tensor("ident32", (128, 128), F32, kind="ExternalInput").ap()
    ident16d = nc.dram_tensor("ident16", (128, 128), F16, kind="ExternalInput").ap()
    wrowd = nc.dram_tensor("wrow", (1, S), F16, kind="ExternalInput").ap()
    onesrowd = nc.dram_tensor("onesrow", (1, 512), F16, kind="ExternalInput").ap()
    ones1x128d = nc.dram_tensor("ones1x128", (1, 128), F16, kind="ExternalInput").ap()
    outd = nc.dram_tensor("out", (PAIRS, S, D), F32, kind="ExternalOutput").ap()

    with tile.TileContext(nc) as tc, ExitStack() as ctx:
        cpool = ctx.enter_context(tc.tile_pool(name="consts", bufs=1))
        ident = cpool.tile([128, 128], F32)
        ident16 = cpool.tile([128, 128], F16)
        wrow = cpool.tile([1, S], F16)
        onesrow = cpool.tile([1, 512], F16)
        ones1x128 = cpool.tile([1, 128], F16)
        nc.sync.dma_start(ident[:], identd)
        nc.sync.dma_start(ident16[:], ident16d)
        nc.sync.dma_start(wrow[:], wrowd)
        nc.sync.dma_start(onesrow[:], onesrowd)
        nc.sync.dma_start(ones1x128[:], ones1x128d)

        inpool = ctx.enter_context(tc.tile_pool(name="inp", bufs=2))
        tpool = ctx.enter_context(tc.tile_pool(name="tposed", bufs=2))
        sapool = ctx.enter_context(tc.tile_pool(name="sa", bufs=2))
        stpool = ctx.enter_context(tc.tile_pool(name="state", bufs=2))
        scr16 = ctx.enter_context(tc.tile_pool(name="scr16", bufs=3))
        rowpool = ctx.enter_context(tc.tile_pool(name="rows", bufs=2))
        bpool = ctx.enter_context(tc.tile_pool(name="stageb", bufs=3))
        opool = ctx.enter_context(tc.tile_pool(name="outs", bufs=2))
        drpool = ctx.enter_context(tc.tile_pool(name="drscratch", bufs=2, space="DRAM"))
        pssmall = ctx.enter_context(tc.tile_pool(name="pssmall", bufs=2, space="PSUM"))
        ps512 = ctx.enter_context(tc.tile_pool(name="ps512", bufs=4, space="PSUM"))
        psbig = ctx.enter_context(tc.tile_pool(name="psbig", bufs=2, space="PSUM"))

        for p in range(PAIRS):
            # ---- load inputs ------------------------------------------------
            qN = inpool.tile([128, QT, D], F32, tag="qN")
            kN = inpool.tile([128, QT, D], F32, tag="kN")
            vN = inpool.tile([128, QT, D], F32, tag="vN")
            nc.sync.dma_start(qN[:], qd[p].rearrange("(t p) d -> p t d", p=128))
            nc.sync.dma_start(kN[:], kd[p].rearrange("(t p) d -> p t d", p=128))
            nc.sync.dma_start(vN[:], vd[p].rearrange("(t p) d -> p t d", p=128))

            # v in f16 with a ones column appended (row 64 of p@V psum = sigma)
            vA = inpool.tile([128, QT, D + 1], F16, tag="vA")
            nc.vector.tensor_copy(vA[:, :, 0:D], vN[:])
            nc.vector.memset(vA[:, :, D:D + 1], 1.0)

            # ---- transpose q,k to [d, s] ------------------------------------
            qT = tpool.tile([64, S], F32R, tag="qT")
            kT = tpool.tile([64, S], F32R, tag="kT")
            for t in range(QT):
                pst = pssmall.tile([64, 128], F32, tag="pssm")
                nc.tensor.transpose(pst[:], qN[:, t, :], ident[:])
                nc.scalar.activation(qT[:, 128 * t:128 * (t + 1)], pst[:], AF.Copy)
                pst2 = pssmall.tile([64, 128], F32, tag="pssm")
                nc.tensor.transpose(pst2[:], kN[:, t, :], ident[:])
                nc.scalar.activation(kT[:, 128 * t:128 * (t + 1)], pst2[:], AF.Copy)

            qbT = tpool.tile([64, S], F16, tag="qbT")
            kbT = tpool.tile([64, S], F16, tag="kbT")
            nc.scalar.activation(qbT[:], qT[:], AF.Sign)
            nc.scalar.activation(kbT[:], kT[:], AF.Sign)

            # ---- layout-A approx scores s[q, k] (f16, exact integers) -------
            sa16 = sapool.tile([128, QT, S], F16, tag="sa16")
            for t in range(QT):
                for h in range(NH):
                    psA = ps512.tile([128, 512], F32, tag="ps512")
                    nc.tensor.matmul(psA[:], qbT[:, 128 * t:128 * (t + 1)],
                                     kbT[:, 512 * h:512 * (h + 1)],
                                     start=True, stop=True)
                    nc.scalar.activation(sa16[:, t, 512 * h:512 * (h + 1)],
                                         psA[:], AF.Copy)

            if KSTAGE <= 1:
                ofin = opool.tile([128, QT, D], F32, tag="ofin")
                for t in range(QT):
                    nc.vector.tensor_copy(ofin[:, t, :], sa16[:, t, 0:D])
                nc.sync.dma_start(outd[p].rearrange("(t p) d -> p t d", p=128),
                                  ofin[:])
                continue

            # ---- phase 1: threshold level bisection -------------------------
            # candidates T(i) = 2i - 65 (odd), i in [0, 128); lo feasible,
            # hi infeasible; power-of-2 widths keep midpoints integral.
            lo = stpool.tile([128, QT], F32, tag="lo")
            hi = stpool.tile([128, QT], F32, tag="hi")
            cnt_hi = stpool.tile([128, QT], F32, tag="cnth")
            nc.vector.memset(lo[:], 0.0)
            nc.vector.memset(hi[:], 128.0)
            nc.vector.memset(cnt_hi[:], 0.0)

            for it in range(7):
                mid = stpool.tile([128, QT], F32, tag="mid")
                nc.vector.tensor_add(mid[:], lo[:], hi[:])
                nc.vector.tensor_scalar_mul(mid[:], mid[:], 0.5)
                tq = stpool.tile([128, QT], F32, tag="tq")
                nc.vector.tensor_scalar(tq[:], mid[:], 2.0, -65.0, OP.mult, OP.add)
                cnt = stpool.tile([128, QT], F32, tag="cnt")
                for t in range(QT):
                    scr = scr16.tile([128, S], F16, tag="scr")
                    nc.vector.tensor_scalar(scr[:], sa16[:, t, :],
                                            tq[:, t:t + 1], None, OP.is_ge,
                                            OP.add, accum_out=cnt[:, t:t + 1])
                feas = stpool.tile([128, QT], mybir.dt.int32, tag="feas")
                nc.vector.tensor_scalar(feas[:], cnt[:], float(KP), None, OP.is_ge)
                lo2 = stpool.tile([128, QT], F32, tag="lo")
                hi2 = stpool.tile([128, QT], F32, tag="hi")
                ch2 = stpool.tile([128, QT], F32, tag="cnth")
                nc.vector.select(lo2[:], feas[:], mid[:], lo[:])
                nc.vector.select(hi2[:], feas[:], hi[:], mid[:])
                nc.vector.select(ch2[:], feas[:], cnt_hi[:], cnt[:])
                lo, hi, cnt_hi = lo2, hi2, ch2

            # t_level = T(lo)+1 = 2*lo - 64 (even);  m = cnt_hi = #(s > t)
            tlev = stpool.tile([128, QT], F32, tag="tlev")
            nc.vector.tensor_scalar(tlev[:], lo[:], 2.0, -64.0, OP.mult, OP.add)
            rm1 = stpool.tile([128, QT], F32, tag="rm1")
            nc.vector.tensor_scalar(rm1[:], cnt_hi[:], -1.0, float(KP - 1),
                                    OP.mult, OP.add)

            if KSTAGE <= 2:
                ofin = opool.tile([128, QT, D], F32, tag="ofin")
                for t in range(QT):
                    nc.vector.tensor_copy(ofin[:, t, 0:1], tlev[:, t:t + 1])
                    nc.vector.tensor_copy(ofin[:, t, 1:2], cnt_hi[:, t:t + 1])
                    nc.vector.memset(ofin[:, t, 2:D], 0.0)
                nc.sync.dma_start(outd[p].rearrange("(t p) d -> p t d", p=128),
                                  ofin[:])
                continue

            # ---- phase 2: tie cutoff index c_q ------------------------------
            ccnt = stpool.tile([128, QT], F32, tag="ccnt")
            for t in range(QT):
                eq = scr16.tile([128, S], F16, tag="eq")
                nc.vector.tensor_scalar(eq[:], sa16[:, t, :],
                                        tlev[:, t:t + 1], None, OP.is_equal)
                pre = scr16.tile([128, S], F16, tag="pre")
                nc.vector.tensor_tensor_scan(pre[:], eq[:], eq[:], 0.0,
                                             OP.add, OP.bypass)
                junk = scr16.tile([128, S], F16, tag="junk")
                nc.vector.tensor_scalar(junk[:], pre[:], rm1[:, t:t + 1], None,
                                        OP.is_le, OP.add,
                                        accum_out=ccnt[:, t:t + 1])

            # tau components: t (f16-exact int) and (1023-c)/1024 (f16-exact)
            t16 = stpool.tile([128, QT], F16, tag="t16")
            nc.vector.tensor_copy(t16[:], tlev[:])
            frac16 = stpool.tile([128, QT], F16, tag="frac16")
            nc.vector.tensor_scalar(frac16[:], ccnt[:], -1.0 / S, (S - 1.0) / S,
                                    OP.mult, OP.add)

            # flatten per-query columns to rows [1, S] (order q = 128*t + p)
            # via a DRAM bounce: SBUF partition-crossing DMAs don't balance.
            trow = rowpool.tile([1, S], F16, tag="trow")
            fracrow = rowpool.tile([1, S], F16, tag="fracrow")
            tdr = drpool.tile([S], F16, tag="tdr")
            fdr = drpool.tile([S], F16, tag="fdr")
            nc.sync.dma_start(tdr[:], t16[:])      # dram linear 8p + t
            nc.sync.dma_start(fdr[:], frac16[:])
            nc.sync.dma_start(trow[0:1, :],
                              tdr[:].rearrange("(p t) -> t p", p=128))
            nc.sync.dma_start(fracrow[0:1, :],
                              fdr[:].rearrange("(p t) -> t p", p=128))

            # tau replicated across partitions: [128, 512] per half
            tausb = []
            for h in range(NH):
                psT = ps512.tile([128, 512], F32, tag="ps512")
                nc.tensor.matmul(psT[:], ones1x128[:],
                                 trow[0:1, 512 * h:512 * (h + 1)],
                                 start=True, stop=False)
                nc.tensor.matmul(psT[:], ones1x128[:],
                                 fracrow[0:1, 512 * h:512 * (h + 1)],
                                 start=False, stop=True)
                tsb = bpool.tile([128, 512], F32, tag="tausb")
                nc.scalar.activation(tsb[:], psT[:], AF.Copy)
                tausb.append(tsb)

            if KSTAGE <= 3:
                ofin = opool.tile([128, QT, D], F32, tag="ofin")
                for t in range(QT):
                    nc.vector.tensor_copy(ofin[:, t, 0:1], tlev[:, t:t + 1])
                    nc.vector.tensor_copy(ofin[:, t, 1:2], ccnt[:, t:t + 1])
                    nc.vector.tensor_copy(ofin[:, t, 2:3], cnt_hi[:, t:t + 1])
                    nc.vector.memset(ofin[:, t, 3:D], 0.0)
                for h in range(NH):
                    nc.vector.tensor_copy(ofin[:, 2 * h, 3:4],
                                          tausb[h][:, 0:1])
                nc.sync.dma_start(outd[p].rearrange("(t p) d -> p t d", p=128),
                                  ofin[:])
                continue

            # ---- stage B: masked softmax attention in [k, q] layout ---------
            psO = []
            for h in range(NH):
                psO_h = psbig.tile([65, 512], F32, tag="psO")
                psO.append(psO_h)

            for kt in range(QT):
                for h in range(NH):
                    psV = ps512.tile([128, 512], F32, tag="ps512")
                    nc.tensor.matmul(psV[:], kbT[:, 128 * kt:128 * (kt + 1)],
                                     qbT[:, 512 * h:512 * (h + 1)],
                                     start=True, stop=False)
                    nc.tensor.matmul(psV[:], wrow[0:1, 128 * kt:128 * (kt + 1)],
                                     onesrow[:], start=False, stop=True)
                    psP = ps512.tile([128, 512], F32, tag="ps512")
                    nc.tensor.matmul(psP[:],
                                     kT[:, 128 * kt:128 * (kt + 1)],
                                     qT[:, 512 * h:512 * (h + 1)],
                                     start=True, stop=True)
                    g16 = bpool.tile([128, 512], F16, tag="g16")
                    nc.vector.tensor_tensor(g16[:], psV[:], tausb[h][:], OP.is_ge)
                    e16 = bpool.tile([128, 512], F16, tag="e16")
                    nc.scalar.activation(e16[:], psP[:], AF.Exp, scale=0.125)
                    p16 = bpool.tile([128, 512], F16, tag="p16")
                    nc.vector.tensor_mul(p16[:], e16[:], g16[:])
                    nc.tensor.matmul(psO[h][:], vA[:, kt, :], p16[:],
                                     start=(kt == 0), stop=(kt == QT - 1))

            if KSTAGE <= 4:
                ofin = opool.tile([128, QT, D], F32, tag="ofin")
                for h in range(NH):
                    nc.scalar.activation(ofin[0:65, 2 * h, :], psO[h][:, 0:D],
                                         AF.Copy)
                nc.vector.memset(ofin[:, 1, :], 0.0)
                nc.vector.memset(ofin[:, 3, :], 0.0)
                nc.vector.memset(ofin[65:128, 0, :], 0.0)
                nc.vector.memset(ofin[65:128, 2, :], 0.0)
                for t in range(4, QT):
                    nc.vector.memset(ofin[:, t, :], 0.0)
                nc.sync.dma_start(outd[p].rearrange("(t p) d -> p t d", p=128),
                                  ofin[:])
                continue

            # ---- normalize + transpose back + store -------------------------
            osb = opool.tile([64, S], F16, tag="osb")
            sgrow = rowpool.tile([1, S], F32, tag="sgrow")
            for h in range(NH):
                nc.scalar.activation(osb[:, 512 * h:512 * (h + 1)],
                                     psO[h][0:64, :], AF.Copy)
                nc.scalar.activation(sgrow[0:1, 512 * h:512 * (h + 1)],
                                     psO[h][64:65, :], AF.Copy)
            sgcol = stpool.tile([128, QT], F32, tag="sgcol")
            sgdr = drpool.tile([S], F32, tag="sgdr")
            nc.sync.dma_start(sgdr[:], sgrow[0:1, :])   # dram linear q-order
            nc.sync.dma_start(sgcol[:],
                              sgdr[:].rearrange("(t p) -> p t", p=128))
            rsg = stpool.tile([128, QT], F32, tag="rsg")
            nc.vector.reciprocal(rsg[:], sgcol[:])

            ofin = opool.tile([128, QT, D], F32, tag="ofin")
            for t in range(QT):
                psB = pssmall.tile([128, 64], F16, tag="pssm")
                nc.tensor.transpose(psB[:], osb[:, 128 * t:128 * (t + 1)],
                                    ident16[0:64, 0:64])
                nc.vector.tensor_scalar(ofin[:, t, :], psB[:],
                                        rsg[:, t:t + 1], None, OP.mult)
            nc.sync.dma_start(outd[p].rearrange("(t p) d -> p t d", p=128),
                              ofin[:])

    nc.compile()
    return nc


_NC = None


def _get_nc():
    global _NC
    if _NC is None:
        _NC = build_program()
    return _NC


def kernel(q, k, v, mask):
    q = np.ascontiguousarray(np.asarray(q, dtype=np.float32))
    k = np.ascontiguousarray(np.asarray(k, dtype=np.float32))
    v = np.ascontiguousarray(np.asarray(v, dtype=np.float32))
    # mask is all-zeros per the problem spec (fill: zeros); the kernel bakes
    # that in (softmax over selected keys is unaffected by adding zeros).
    assert np.all(np.asarray(mask) == 0.0), "kernel assumes zero mask"

    qf = q.reshape(B * H, S, D)
    kf = k.reshape(B * H, S, D)
    vf = v.reshape(B * H, S, D)
    ident32, ident16, wrow, onesrow, ones1x128 = _consts()

    in_maps = []
    for c in range(NCORES):
        sl = slice(c * PAIRS, (c + 1) * PAIRS)
        in_maps.append({
            "q_in": qf[sl], "k_in": kf[sl], "v_in": vf[sl],
            "ident32": ident32, "ident16": ident16, "wrow": wrow,
            "onesrow": onesrow, "ones1x128": ones1x128,
        })

    nc = _get_nc()
    res = run_bass_kernel_spmd(nc, in_maps, core_ids=list(range(NCORES)))
    outs = [res.results[c]["out"] for c in range(NCORES)]
    out = np.concatenate(outs, axis=0).reshape(B, H, S, D)
    return out.astype(np.float32)



# revision 8
# speedup vs baseline: 1.4048x; 1.4048x over previous
"""Binary-approximate sparse attention on 8 Trainium2 NeuronCores.

Reference semantics (per batch b, head h, query q):
  s      = sign(q) . sign(k)            -- integer scores in [-64, 64], even
  top-k  = 102 largest s, ties broken toward LOWER key index (jax.lax.top_k)
  out    = softmax over the precise scores (q.k/8) of the selected keys @ v

v2: work spread across DVE + ACT + Pool engines.
  - 6-step bisection over the 65 even score levels (window [0,64] on the odd
    candidate lattice T(i) = 2i - 65); count passes split between the vector
    engine (fused is_ge+accum) and the scalar engine (Sign activation with
    per-partition bias, free-dim accumulator: cnt = (acc + S)/2).
  - tie-cutoff count (#(prefix <= r-1)) on the scalar engine the same way.
  - no wrow matmul in stage B: the per-key augmentation w_k = (S-1-k)/S is
    split into a per-tile scalar a_kt (folded into the DVE compare via
    scalar_tensor_tensor) and a per-partition ramp kp/S (folded into the
    broadcast tau tile).
  - PSUM->SBUF copies and the p = e*g multiply run on the gpsimd engine.
"""

import numpy as np

from contextlib import ExitStack

import concourse.bacc as bacc
import concourse.bass as bass
import concourse.mybir as mybir
import concourse.tile as tile
from concourse.bass_utils import run_bass_kernel_spmd

B, H, S, D = 2, 12, 1024, 64
NCORES = 8
PAIRS = (B * H) // NCORES          # (b,h) pairs per core
KP = 102                           # top-k
QT = S // 128                      # 128-row tiles per axis
NH = S // 512                      # 512-col halves
NITER = 6                          # bisection steps over window [0, 64]
ACT_TILES = 4                      # count tiles 0..4 on ACT, 5..7 on DVE
USE_POOL = True                    # p = e*g multiply on the gpsimd engine

F32 = mybir.dt.float32
F32R = mybir.dt.float32r
F16 = mybir.dt.float16
AF = mybir.ActivationFunctionType
OP = mybir.AluOpType


def _register_tie_cut():
    """Custom DVE op fusing phase 2 into one pass per tile:
      pre = cumsum(s == tlev); out = (pre < r); accum = #(pre < r) = c,
    the 0-based key index of the r-th tie (ties broken toward lower index).
    Replaces eq (tensor_scalar) + prefix scan + le-count, 3 full passes."""
    import concourse.dve_ops as dve_ops
    from concourse.dve_spec import Spec, Src0, C0, C1, AluOp, eq, scan, lower
    from concourse.dve_uop import DveOpSpec

    name = "TIE_CUT_ANT"
    if any(o.name == name for o in dve_ops.OPS):
        return next(o for o in dve_ops.OPS if o.name == name)

    def _ref(in0, in1, c0, c1, c2):
        pre = np.cumsum(in0.astype(np.float32) == c0, axis=1)
        out = (pre < c1).astype(np.float32)
        return out, out.sum(axis=1, keepdims=True)

    spec = Spec(body=scan(AluOp.ADD, eq(Src0, C0)) < C1, reference=_ref,
                accum=AluOp.ADD)
    row = dve_ops._CUSTOM_DVE_ROW_BASE + len(dve_ops.OPS)
    assert row < 0x20
    uops = lower(spec, ver="v3")
    sha3 = DveOpSpec(name=name, opcode=row, uops=uops,
                     rd1_en=dve_ops.has_src1(spec)).sha("v3")
    op = dve_ops.DveOp(name, spec, subdim=False, uops_sha={"v3": sha3})
    dve_ops.OPS.append(op)
    dve_ops._SUB_OPCODE_FOR_NAME[name] = row
    dve_ops.CUSTOM_DVE_SPECS[name] = spec
    return op



def _consts():
    ident32 = np.eye(128, dtype=np.float32)
    ident16 = np.eye(128, dtype=np.float16)
    onesrow = np.ones((1, 512), dtype=np.float16)
    ones1x128 = np.ones((1, 128), dtype=np.float16)
    wrow = (((S - 1) - np.arange(S, dtype=np.float32)) / S).astype(np.float16)[None, :]
    return ident32, ident16, onesrow, ones1x128, wrow


def make_in_maps(qf, kf, vf):
    ident32, ident16, onesrow, ones1x128, wrow = _consts()
    in_maps = []
    for c in range(NCORES):
        sl = slice(c * PAIRS, (c + 1) * PAIRS)
        in_maps.append({
            "q_in": qf[sl], "k_in": kf[sl], "v_in": vf[sl],
            "ident32": ident32, "ident16": ident16,
            "onesrow": onesrow, "ones1x128": ones1x128, "wrow": wrow,
        })
    return in_maps


def build_program():
    TIE_CUT = _register_tie_cut()
    nc = bacc.Bacc("TRN2", target_bir_lowering=False, debug=False,
                   num_devices=NCORES)

    qd = nc.dram_tensor("q_in", (PAIRS, S, D), F32, kind="ExternalInput").ap()
    kd = nc.dram_tensor("k_in", (PAIRS, S, D), F32, kind="ExternalInput").ap()
    vd = nc.dram_tensor("v_in", (PAIRS, S, D), F32, kind="ExternalInput").ap()
    identd = nc.dram_tensor("ident32", (128, 128), F32, kind="ExternalInput").ap()
    ident16d = nc.dram_tensor("ident16", (128, 128), F16, kind="ExternalInput").ap()
    onesrowd = nc.dram_tensor("onesrow", (1, 512), F16, kind="ExternalInput").ap()
    ones1x128d = nc.dram_tensor("ones1x128", (1, 128), F16, kind="ExternalInput").ap()
    wrowd = nc.dram_tensor("wrow", (1, S), F16, kind="ExternalInput").ap()
    outd = nc.dram_tensor("out", (PAIRS, S, D), F32, kind="ExternalOutput").ap()

    with tile.TileContext(nc) as tc, ExitStack() as ctx:
        cpool = ctx.enter_context(tc.tile_pool(name="consts", bufs=1))
        ident = cpool.tile([128, 128], F32)
        ident16 = cpool.tile([128, 128], F16)
        onesrow = cpool.tile([1, 512], F16)
        ones1x128 = cpool.tile([1, 128], F16)
        wrow = cpool.tile([1, S], F16)
        nc.sync.dma_start(ident[:], identd)
        nc.sync.dma_start(ident16[:], ident16d)
        nc.sync.dma_start(onesrow[:], onesrowd)
        nc.sync.dma_start(ones1x128[:], ones1x128d)
        nc.sync.dma_start(wrow[:], wrowd)

        inpool = ctx.enter_context(tc.tile_pool(name="inp", bufs=2))
        tpool = ctx.enter_context(tc.tile_pool(name="tposed", bufs=2))
        sapool = ctx.enter_context(tc.tile_pool(name="sa", bufs=2))
        stpool = ctx.enter_context(tc.tile_pool(name="state", bufs=2))
        scr16 = ctx.enter_context(tc.tile_pool(name="scr16", bufs=3))
        jpool = ctx.enter_context(tc.tile_pool(name="junk", bufs=2))
        rowpool = ctx.enter_context(tc.tile_pool(name="rows", bufs=2))
        bpool = ctx.enter_context(tc.tile_pool(name="stageb", bufs=3))
        opool = ctx.enter_context(tc.tile_pool(name="outs", bufs=2))
        drpool = ctx.enter_context(tc.tile_pool(name="drscratch", bufs=2, space="DRAM"))
        pssmall = ctx.enter_context(tc.tile_pool(name="pssmall", bufs=2, space="PSUM"))
        ps512 = ctx.enter_context(tc.tile_pool(name="ps512", bufs=4, space="PSUM"))
        psbig = ctx.enter_context(tc.tile_pool(name="psbig", bufs=2, space="PSUM"))

        for p in range(PAIRS):
            # ---- load inputs ------------------------------------------------
            qN = inpool.tile([128, QT, D], F32, tag="qN")
            kN = inpool.tile([128, QT, D], F32, tag="kN")
            vN = inpool.tile([128, QT, D], F32, tag="vN")
            nc.sync.dma_start(qN[:], qd[p].rearrange("(t p) d -> p t d", p=128))
            nc.sync.dma_start(kN[:], kd[p].rearrange("(t p) d -> p t d", p=128))
            nc.sync.dma_start(vN[:], vd[p].rearrange("(t p) d -> p t d", p=128))

            # v in f16 with a ones column appended (row 64 of p@V psum = sigma)
            vA = inpool.tile([128, QT, D + 1], F16, tag="vA")
            nc.scalar.copy(vA[:, :, 0:D], vN[:])
            nc.vector.memset(vA[:, :, D:D + 1], 1.0)

            # ---- transpose q,k to [d, s] ------------------------------------
            qT = tpool.tile([64, S], F32R, tag="qT")
            kT = tpool.tile([64, S], F32R, tag="kT")
            for t in range(QT):
                pst = pssmall.tile([64, 128], F32, tag="pssm")
                nc.tensor.transpose(pst[:], qN[:, t, :], ident[:])
                nc.scalar.activation(qT[:, 128 * t:128 * (t + 1)], pst[:], AF.Copy)
                pst2 = pssmall.tile([64, 128], F32, tag="pssm")
                nc.tensor.transpose(pst2[:], kN[:, t, :], ident[:])
                nc.scalar.activation(kT[:, 128 * t:128 * (t + 1)], pst2[:], AF.Copy)

            qbT = tpool.tile([64, S], F16, tag="qbT")
            kbT = tpool.tile([64, S], F16, tag="kbT")
            nc.scalar.activation(qbT[:], qT[:], AF.Sign)
            nc.scalar.activation(kbT[:], kT[:], AF.Sign)

            # ---- layout-A approx scores s[q, k] (f16, exact integers) -------
            sa16 = sapool.tile([128, QT, S], F16, tag="sa16")
            for t in range(QT):
                for h in range(NH):
                    psA = ps512.tile([128, 512], F32, tag="ps512")
                    nc.tensor.matmul(psA[:], qbT[:, 128 * t:128 * (t + 1)],
                                     kbT[:, 512 * h:512 * (h + 1)],
                                     start=True, stop=True)
                    nc.vector.tensor_copy(sa16[:, t, 512 * h:512 * (h + 1)],
                                          psA[:])

            # ---- phase 1: threshold level bisection -------------------------
            # candidates T(i) = 2i - 65 (odd), i in [0, 64]; lo feasible,
            # hi infeasible; power-of-2 widths keep midpoints integral.
            lo = stpool.tile([128, QT], F32, tag="lo")
            hi = stpool.tile([128, QT], F32, tag="hi")
            cnt_hi = stpool.tile([128, QT], F32, tag="cnth")
            nc.vector.memset(lo[:], 0.0)
            nc.vector.memset(hi[:], 64.0)
            nc.vector.memset(cnt_hi[:], 0.0)

            for it in range(NITER):
                # T(mid) = 2*(lo+hi)/2 - 65 = lo + hi - 65
                tq = stpool.tile([128, QT], F32, tag="tq")
                nc.vector.scalar_tensor_tensor(tq[:], lo[:], -65.0, hi[:],
                                               OP.add, OP.add)
                ntq = stpool.tile([128, QT], F32, tag="ntq")
                nc.vector.tensor_scalar(ntq[:], tq[:], -1.0, None, OP.mult)
                cnt = stpool.tile([128, QT], F32, tag="cnt")
                acc = stpool.tile([128, QT], F32, tag="acc")
                for t in range(QT):
                    if t < ACT_TILES:
                        ja = jpool.tile([128, S], F16, tag="ja")
                        nc.scalar.activation(ja[:], sa16[:, t, :], AF.Sign,
                                             bias=ntq[:, t:t + 1],
                                             accum_out=acc[:, t:t + 1])
                    else:
                        jd = jpool.tile([128, S], F16, tag="jd")
                        nc.vector.tensor_scalar(jd[:], sa16[:, t, :],
                                                tq[:, t:t + 1], None, OP.is_ge,
                                                OP.add,
                                                accum_out=cnt[:, t:t + 1])
                # ACT columns: cnt = acc/2 + S/2
                nc.vector.tensor_scalar(cnt[:, 0:ACT_TILES],
                                        acc[:, 0:ACT_TILES], 0.5, S / 2.0,
                                        OP.mult, OP.add)
                feas = stpool.tile([128, QT], mybir.dt.int32, tag="feas")
                nc.vector.tensor_scalar(feas[:], cnt[:], float(KP), None,
                                        OP.is_ge)
                mid = stpool.tile([128, QT], F32, tag="mid")
                nc.vector.tensor_scalar(mid[:], tq[:], 65.0, 0.5, OP.add,
                                        OP.mult)
                lo2 = stpool.tile([128, QT], F32, tag="lo")
                hi2 = stpool.tile([128, QT], F32, tag="hi")
                ch2 = stpool.tile([128, QT], F32, tag="cnth")
                nc.vector.select(lo2[:], feas[:], mid[:], lo[:])
                nc.vector.select(hi2[:], feas[:], hi[:], mid[:])
                nc.vector.select(ch2[:], feas[:], cnt_hi[:], cnt[:])
                lo, hi, cnt_hi = lo2, hi2, ch2

            # t_level = T(lo)+1 = 2*lo - 64 (even);  m = cnt_hi = #(s > t)
            tlev = stpool.tile([128, QT], F32, tag="tlev")
            nc.vector.tensor_scalar(tlev[:], lo[:], 2.0, -64.0, OP.mult, OP.add)
            # r = KP - cnt_hi  (rank of the last tie to keep)
            rq = stpool.tile([128, QT], F32, tag="rq")
            nc.vector.tensor_scalar(rq[:], cnt_hi[:], -1.0, float(KP),
                                    OP.mult, OP.add)

            # ---- phase 2: tie cutoff index c_q (one fused pass per tile) ----
            ccnt = stpool.tile([128, QT], F32, tag="ccnt")
            for t in range(QT):
                jt = jpool.tile([128, S], F16, tag="jd")
                nc.vector._custom_dve(TIE_CUT, out=jt[:], in0=sa16[:, t, :],
                                      s0=tlev[:, t:t + 1], s1=rq[:, t:t + 1],
                                      accum_out=ccnt[:, t:t + 1])

            # tau components: t (f16-exact int) and (S-1-c)/S (f16-exact)
            t16 = stpool.tile([128, QT], F16, tag="t16")
            nc.vector.tensor_copy(t16[:], tlev[:])
            frac16 = stpool.tile([128, QT], F16, tag="frac16")
            nc.vector.tensor_scalar(frac16[:], ccnt[:], -1.0 / S,
                                    (S - 1.0) / S, OP.mult, OP.add)

            # flatten per-query columns to rows [1, S] (order q = 128*t + p)
            # via a DRAM bounce: SBUF partition-crossing DMAs don't balance.
            trow = rowpool.tile([1, S], F16, tag="trow")
            fracrow = rowpool.tile([1, S], F16, tag="fracrow")
            tdr = drpool.tile([S], F16, tag="tdr")
            fdr = drpool.tile([S], F16, tag="fdr")
            nc.sync.dma_start(tdr[:], t16[:])      # dram linear 8p + t
            nc.sync.dma_start(fdr[:], frac16[:])
            nc.sync.dma_start(trow[0:1, :],
                              tdr[:].rearrange("(p t) -> t p", p=128))
            nc.sync.dma_start(fracrow[0:1, :],
                              fdr[:].rearrange("(p t) -> t p", p=128))

            # tau = tlev_q + frac_q, replicated across partitions [128, 512]
            tausb = []
            for h in range(NH):
                psT = ps512.tile([128, 512], F32, tag="ps512")
                nc.tensor.matmul(psT[:], ones1x128[:],
                                 trow[0:1, 512 * h:512 * (h + 1)],
                                 start=True, stop=False)
                nc.tensor.matmul(psT[:], ones1x128[:],
                                 fracrow[0:1, 512 * h:512 * (h + 1)],
                                 start=False, stop=True)
                tsb = bpool.tile([128, 512], F32, tag="tausb")
                nc.scalar.activation(tsb[:], psT[:], AF.Copy)
                tausb.append(tsb)

            # ---- stage B: masked softmax attention in [k, q] layout ---------
            psO = []
            for h in range(NH):
                psO_h = psbig.tile([65, 512], F32, tag="psO")
                psO.append(psO_h)

            for kt in range(QT):
                for h in range(NH):
                    psV = ps512.tile([128, 512], F32, tag="ps512")
                    nc.tensor.matmul(psV[:], kbT[:, 128 * kt:128 * (kt + 1)],
                                     qbT[:, 512 * h:512 * (h + 1)],
                                     start=True, stop=False)
                    nc.tensor.matmul(psV[:], wrow[0:1, 128 * kt:128 * (kt + 1)],
                                     onesrow[:], start=False, stop=True)
                    psP = ps512.tile([128, 512], F32, tag="ps512")
                    nc.tensor.matmul(psP[:],
                                     kT[:, 128 * kt:128 * (kt + 1)],
                                     qT[:, 512 * h:512 * (h + 1)],
                                     start=True, stop=True)
                    g16 = bpool.tile([128, 512], F16, tag="g16")
                    nc.vector.tensor_tensor(g16[:], psV[:], tausb[h][:],
                                            OP.is_ge)
                    e16 = bpool.tile([128, 512], F16, tag="e16")
                    nc.scalar.activation(e16[:], psP[:], AF.Exp, scale=0.125)
                    p16 = bpool.tile([128, 512], F16, tag="p16")
                    peng = nc.gpsimd if USE_POOL else nc.vector
                    peng.tensor_tensor(p16[:], e16[:], g16[:], OP.mult)
                    nc.tensor.matmul(psO[h][:], vA[:, kt, :], p16[:],
                                     start=(kt == 0), stop=(kt == QT - 1))

            # ---- normalize + transpose back + store -------------------------
            osb = opool.tile([64, S], F16, tag="osb")
            sgrow = rowpool.tile([1, S], F32, tag="sgrow")
            for h in range(NH):
                nc.scalar.activation(osb[:, 512 * h:512 * (h + 1)],
                                     psO[h][0:64, :], AF.Copy)
                nc.scalar.activation(sgrow[0:1, 512 * h:512 * (h + 1)],
                                     psO[h][64:65, :], AF.Copy)
            sgcol = stpool.tile([128, QT], F32, tag="sgcol")
            sgdr = drpool.tile([S], F32, tag="sgdr")
            nc.sync.dma_start(sgdr[:], sgrow[0:1, :])   # dram linear q-order
            nc.sync.dma_start(sgcol[:],
                              sgdr[:].rearrange("(t p) -> p t", p=128))
            rsg = stpool.tile([128, QT], F32, tag="rsg")
            nc.vector.reciprocal(rsg[:], sgcol[:])

            ofin = opool.tile([128, QT, D], F32, tag="ofin")
            for t in range(QT):
                psB = pssmall.tile([128, 64], F16, tag="pssm")
                nc.tensor.transpose(psB[:], osb[:, 128 * t:128 * (t + 1)],
                                    ident16[0:64, 0:64])
                nc.scalar.activation(ofin[:, t, :], psB[:], AF.Copy,
                                     scale=rsg[:, t:t + 1])
            nc.sync.dma_start(outd[p].rearrange("(t p) d -> p t d", p=128),
                              ofin[:])

    nc.compile()
    return nc


_NC = None


def _get_nc():
    global _NC
    if _NC is None:
        _NC = build_program()
    return _NC


def kernel(q, k, v, mask):
    q = np.ascontiguousarray(np.asarray(q, dtype=np.float32))
    k = np.ascontiguousarray(np.asarray(k, dtype=np.float32))
    v = np.ascontiguousarray(np.asarray(v, dtype=np.float32))
    # mask is all-zeros per the problem spec (fill: zeros); the kernel bakes
    # that in (softmax over selected keys is unaffected by adding zeros).
    assert np.all(np.asarray(mask) == 0.0), "kernel assumes zero mask"

    qf = q.reshape(B * H, S, D)
    kf = k.reshape(B * H, S, D)
    vf = v.reshape(B * H, S, D)
    in_maps = make_in_maps(qf, kf, vf)

    nc = _get_nc()
    res = run_bass_kernel_spmd(nc, in_maps, core_ids=list(range(NCORES)))
    outs = [res.results[c]["out"] for c in range(NCORES)]
    out = np.concatenate(outs, axis=0).reshape(B, H, S, D)
    return out.astype(np.float32)


# revision 9
# speedup vs baseline: 1.4303x; 1.0182x over previous
"""Binary-approximate sparse attention on 8 Trainium2 NeuronCores.

Reference semantics (per batch b, head h, query q):
  s      = sign(q) . sign(k)            -- integer scores in [-64, 64], even
  top-k  = 102 largest s, ties broken toward LOWER key index (jax.lax.top_k)
  out    = softmax over the precise scores (q.k/8) of the selected keys @ v

v2: work spread across DVE + ACT + Pool engines.
  - 6-step bisection over the 65 even score levels (window [0,64] on the odd
    candidate lattice T(i) = 2i - 65); count passes split between the vector
    engine (fused is_ge+accum) and the scalar engine (Sign activation with
    per-partition bias, free-dim accumulator: cnt = (acc + S)/2).
  - tie-cutoff count (#(prefix <= r-1)) on the scalar engine the same way.
  - no wrow matmul in stage B: the per-key augmentation w_k = (S-1-k)/S is
    split into a per-tile scalar a_kt (folded into the DVE compare via
    scalar_tensor_tensor) and a per-partition ramp kp/S (folded into the
    broadcast tau tile).
  - PSUM->SBUF copies and the p = e*g multiply run on the gpsimd engine.
"""

import numpy as np

from contextlib import ExitStack

import concourse.bacc as bacc
import concourse.bass as bass
import concourse.mybir as mybir
import concourse.tile as tile
from concourse.bass_utils import run_bass_kernel_spmd

B, H, S, D = 2, 12, 1024, 64
NCORES = 8
PAIRS = (B * H) // NCORES          # (b,h) pairs per core
KP = 102                           # top-k
QT = S // 128                      # 128-row tiles per axis
NH = S // 512                      # 512-col halves
NITER = 6                          # bisection steps over window [0, 64]
ACT_TILES = 4                      # count tiles 0..4 on ACT, 5..7 on DVE
USE_POOL = False                   # p = e*g multiply on the gpsimd engine

F32 = mybir.dt.float32
F32R = mybir.dt.float32r
F16 = mybir.dt.float16
AF = mybir.ActivationFunctionType
OP = mybir.AluOpType


def _register_tie_cut():
    """Custom DVE op fusing phase 2 into one pass per tile:
      pre = cumsum(s == tlev); out = (pre < r); accum = #(pre < r) = c,
    the 0-based key index of the r-th tie (ties broken toward lower index).
    Replaces eq (tensor_scalar) + prefix scan + le-count, 3 full passes."""
    import concourse.dve_ops as dve_ops
    from concourse.dve_spec import Spec, Src0, C0, C1, AluOp, eq, scan, lower
    from concourse.dve_uop import DveOpSpec

    name = "TIE_CUT_ANT"
    if any(o.name == name for o in dve_ops.OPS):
        return next(o for o in dve_ops.OPS if o.name == name)

    def _ref(in0, in1, c0, c1, c2):
        pre = np.cumsum(in0.astype(np.float32) == c0, axis=1)
        out = (pre < c1).astype(np.float32)
        return out, out.sum(axis=1, keepdims=True)

    spec = Spec(body=scan(AluOp.ADD, eq(Src0, C0)) < C1, reference=_ref,
                accum=AluOp.ADD)
    row = dve_ops._CUSTOM_DVE_ROW_BASE + len(dve_ops.OPS)
    assert row < 0x20
    uops = lower(spec, ver="v3")
    sha3 = DveOpSpec(name=name, opcode=row, uops=uops,
                     rd1_en=dve_ops.has_src1(spec)).sha("v3")
    op = dve_ops.DveOp(name, spec, subdim=False, uops_sha={"v3": sha3})
    dve_ops.OPS.append(op)
    dve_ops._SUB_OPCODE_FOR_NAME[name] = row
    dve_ops.CUSTOM_DVE_SPECS[name] = spec
    return op



def _consts():
    ident32 = np.eye(128, dtype=np.float32)
    ident16 = np.eye(128, dtype=np.float16)
    onesrow = np.ones((1, 512), dtype=np.float16)
    ones1x128 = np.ones((1, 128), dtype=np.float16)
    wrow = (((S - 1) - np.arange(S, dtype=np.float32)) / S).astype(np.float16)[None, :]
    return ident32, ident16, onesrow, ones1x128, wrow


def make_in_maps(qf, kf, vf):
    ident32, ident16, onesrow, ones1x128, wrow = _consts()
    in_maps = []
    for c in range(NCORES):
        sl = slice(c * PAIRS, (c + 1) * PAIRS)
        in_maps.append({
            "q_in": qf[sl], "k_in": kf[sl], "v_in": vf[sl],
            "ident32": ident32, "ident16": ident16,
            "onesrow": onesrow, "ones1x128": ones1x128, "wrow": wrow,
        })
    return in_maps


def build_program():
    TIE_CUT = _register_tie_cut()
    nc = bacc.Bacc("TRN2", target_bir_lowering=False, debug=False,
                   num_devices=NCORES)

    qd = nc.dram_tensor("q_in", (PAIRS, S, D), F32, kind="ExternalInput").ap()
    kd = nc.dram_tensor("k_in", (PAIRS, S, D), F32, kind="ExternalInput").ap()
    vd = nc.dram_tensor("v_in", (PAIRS, S, D), F32, kind="ExternalInput").ap()
    identd = nc.dram_tensor("ident32", (128, 128), F32, kind="ExternalInput").ap()
    ident16d = nc.dram_tensor("ident16", (128, 128), F16, kind="ExternalInput").ap()
    onesrowd = nc.dram_tensor("onesrow", (1, 512), F16, kind="ExternalInput").ap()
    ones1x128d = nc.dram_tensor("ones1x128", (1, 128), F16, kind="ExternalInput").ap()
    wrowd = nc.dram_tensor("wrow", (1, S), F16, kind="ExternalInput").ap()
    outd = nc.dram_tensor("out", (PAIRS, S, D), F32, kind="ExternalOutput").ap()

    with tile.TileContext(nc) as tc, ExitStack() as ctx:
        cpool = ctx.enter_context(tc.tile_pool(name="consts", bufs=1))
        ident = cpool.tile([128, 128], F32)
        ident16 = cpool.tile([128, 128], F16)
        onesrow = cpool.tile([1, 512], F16)
        ones1x128 = cpool.tile([1, 128], F16)
        wrow = cpool.tile([1, S], F16)
        nc.sync.dma_start(ident[:], identd)
        nc.sync.dma_start(ident16[:], ident16d)
        nc.sync.dma_start(onesrow[:], onesrowd)
        nc.sync.dma_start(ones1x128[:], ones1x128d)
        nc.sync.dma_start(wrow[:], wrowd)

        inpool = ctx.enter_context(tc.tile_pool(name="inp", bufs=3))
        tpool = ctx.enter_context(tc.tile_pool(name="tposed", bufs=3))
        sapool = ctx.enter_context(tc.tile_pool(name="sa", bufs=3))
        stpool = ctx.enter_context(tc.tile_pool(name="state", bufs=3))
        jpool = ctx.enter_context(tc.tile_pool(name="junk", bufs=3))
        rowpool = ctx.enter_context(tc.tile_pool(name="rows", bufs=3))
        bpool = ctx.enter_context(tc.tile_pool(name="stageb", bufs=4))
        opool = ctx.enter_context(tc.tile_pool(name="outs", bufs=3))
        drpool = ctx.enter_context(tc.tile_pool(name="drscratch", bufs=3, space="DRAM"))
        pssmall = ctx.enter_context(tc.tile_pool(name="pssmall", bufs=2, space="PSUM"))
        ps512 = ctx.enter_context(tc.tile_pool(name="ps512", bufs=4, space="PSUM"))
        psbig = ctx.enter_context(tc.tile_pool(name="psbig", bufs=2, space="PSUM"))

        for p in range(PAIRS):
            # ---- load inputs ------------------------------------------------
            qN = inpool.tile([128, QT, D], F32, tag="qN")
            kN = inpool.tile([128, QT, D], F32, tag="kN")
            vN = inpool.tile([128, QT, D], F32, tag="vN")
            nc.sync.dma_start(qN[:], qd[p].rearrange("(t p) d -> p t d", p=128))
            nc.sync.dma_start(kN[:], kd[p].rearrange("(t p) d -> p t d", p=128))
            nc.sync.dma_start(vN[:], vd[p].rearrange("(t p) d -> p t d", p=128))

            # v in f16 with a ones column appended (row 64 of p@V psum = sigma)
            vA = inpool.tile([128, QT, D + 1], F16, tag="vA")
            nc.scalar.copy(vA[:, :, 0:D], vN[:])
            nc.vector.memset(vA[:, :, D:D + 1], 1.0)

            # ---- transpose q,k to [d, s] ------------------------------------
            qT = tpool.tile([64, S], F32R, tag="qT")
            kT = tpool.tile([64, S], F32R, tag="kT")
            for t in range(QT):
                pst = pssmall.tile([64, 128], F32, tag="pssm")
                nc.tensor.transpose(pst[:], qN[:, t, :], ident[:])
                nc.scalar.activation(qT[:, 128 * t:128 * (t + 1)], pst[:], AF.Copy)
                pst2 = pssmall.tile([64, 128], F32, tag="pssm")
                nc.tensor.transpose(pst2[:], kN[:, t, :], ident[:])
                nc.scalar.activation(kT[:, 128 * t:128 * (t + 1)], pst2[:], AF.Copy)

            qbT = tpool.tile([64, S], F16, tag="qbT")
            kbT = tpool.tile([64, S], F16, tag="kbT")
            nc.scalar.activation(qbT[:], qT[:], AF.Sign)
            nc.scalar.activation(kbT[:], kT[:], AF.Sign)

            # ---- layout-A approx scores s[q, k] (f16, exact integers) -------
            sa16 = sapool.tile([128, QT, S], F16, tag="sa16")
            for t in range(QT):
                for h in range(NH):
                    psA = ps512.tile([128, 512], F32, tag="ps512")
                    nc.tensor.matmul(psA[:], qbT[:, 128 * t:128 * (t + 1)],
                                     kbT[:, 512 * h:512 * (h + 1)],
                                     start=True, stop=True)
                    nc.vector.tensor_copy(sa16[:, t, 512 * h:512 * (h + 1)],
                                          psA[:])

            # ---- phase 1: threshold level bisection -------------------------
            # candidates T(i) = 2i - 65 (odd), i in [0, 64]; lo feasible,
            # hi infeasible; power-of-2 widths keep midpoints integral.
            lo = stpool.tile([128, QT], F32, tag="lo")
            hi = stpool.tile([128, QT], F32, tag="hi")
            cnt_hi = stpool.tile([128, QT], F32, tag="cnth")
            nc.vector.memset(lo[:], 0.0)
            nc.vector.memset(hi[:], 64.0)
            nc.vector.memset(cnt_hi[:], 0.0)

            for it in range(NITER):
                # T(mid) = 2*(lo+hi)/2 - 65 = lo + hi - 65
                tq = stpool.tile([128, QT], F32, tag="tq")
                nc.vector.scalar_tensor_tensor(tq[:], lo[:], -65.0, hi[:],
                                               OP.add, OP.add)
                ntq = stpool.tile([128, QT], F32, tag="ntq")
                nc.vector.tensor_scalar(ntq[:], tq[:], -1.0, None, OP.mult)
                cnt = stpool.tile([128, QT], F32, tag="cnt")
                acc = stpool.tile([128, QT], F32, tag="acc")
                for t in range(QT):
                    if t < ACT_TILES:
                        ja = jpool.tile([128, S], F16, tag="ja")
                        nc.scalar.activation(ja[:], sa16[:, t, :], AF.Sign,
                                             bias=ntq[:, t:t + 1],
                                             accum_out=acc[:, t:t + 1])
                    else:
                        jd = jpool.tile([128, S], F16, tag="jd")
                        nc.vector.tensor_scalar(jd[:], sa16[:, t, :],
                                                tq[:, t:t + 1], None, OP.is_ge,
                                                OP.add,
                                                accum_out=cnt[:, t:t + 1])
                # ACT columns: cnt = acc/2 + S/2
                nc.vector.tensor_scalar(cnt[:, 0:ACT_TILES],
                                        acc[:, 0:ACT_TILES], 0.5, S / 2.0,
                                        OP.mult, OP.add)
                feas = stpool.tile([128, QT], mybir.dt.int32, tag="feas")
                nc.vector.tensor_scalar(feas[:], cnt[:], float(KP), None,
                                        OP.is_ge)
                mid = stpool.tile([128, QT], F32, tag="mid")
                nc.vector.tensor_scalar(mid[:], tq[:], 65.0, 0.5, OP.add,
                                        OP.mult)
                lo2 = stpool.tile([128, QT], F32, tag="lo")
                hi2 = stpool.tile([128, QT], F32, tag="hi")
                ch2 = stpool.tile([128, QT], F32, tag="cnth")
                nc.vector.select(lo2[:], feas[:], mid[:], lo[:])
                nc.vector.select(hi2[:], feas[:], hi[:], mid[:])
                nc.vector.select(ch2[:], feas[:], cnt_hi[:], cnt[:])
                lo, hi, cnt_hi = lo2, hi2, ch2

            # t_level = T(lo)+1 = 2*lo - 64 (even);  m = cnt_hi = #(s > t)
            tlev = stpool.tile([128, QT], F32, tag="tlev")
            nc.vector.tensor_scalar(tlev[:], lo[:], 2.0, -64.0, OP.mult, OP.add)
            # r = KP - cnt_hi  (rank of the last tie to keep)
            rq = stpool.tile([128, QT], F32, tag="rq")
            nc.vector.tensor_scalar(rq[:], cnt_hi[:], -1.0, float(KP),
                                    OP.mult, OP.add)

            # ---- phase 2: tie cutoff index c_q (one fused pass per tile) ----
            ccnt = stpool.tile([128, QT], F32, tag="ccnt")
            for t in range(QT):
                jt = jpool.tile([128, S], F16, tag="jd")
                nc.vector._custom_dve(TIE_CUT, out=jt[:], in0=sa16[:, t, :],
                                      s0=tlev[:, t:t + 1], s1=rq[:, t:t + 1],
                                      accum_out=ccnt[:, t:t + 1])

            # tau components: t (f16-exact int) and (S-1-c)/S (f16-exact)
            t16 = stpool.tile([128, QT], F16, tag="t16")
            nc.vector.tensor_copy(t16[:], tlev[:])
            frac16 = stpool.tile([128, QT], F16, tag="frac16")
            nc.vector.tensor_scalar(frac16[:], ccnt[:], -1.0 / S,
                                    (S - 1.0) / S, OP.mult, OP.add)

            # flatten per-query columns to rows [1, S] (order q = 128*t + p)
            # via a DRAM bounce: SBUF partition-crossing DMAs don't balance.
            trow = rowpool.tile([1, S], F16, tag="trow")
            fracrow = rowpool.tile([1, S], F16, tag="fracrow")
            tdr = drpool.tile([S], F16, tag="tdr")
            fdr = drpool.tile([S], F16, tag="fdr")
            nc.sync.dma_start(tdr[:], t16[:])      # dram linear 8p + t
            nc.sync.dma_start(fdr[:], frac16[:])
            nc.sync.dma_start(trow[0:1, :],
                              tdr[:].rearrange("(p t) -> t p", p=128))
            nc.sync.dma_start(fracrow[0:1, :],
                              fdr[:].rearrange("(p t) -> t p", p=128))

            # tau = tlev_q + frac_q, replicated across partitions [128, 512]
            tausb = []
            for h in range(NH):
                psT = ps512.tile([128, 512], F32, tag="ps512")
                nc.tensor.matmul(psT[:], ones1x128[:],
                                 trow[0:1, 512 * h:512 * (h + 1)],
                                 start=True, stop=False)
                nc.tensor.matmul(psT[:], ones1x128[:],
                                 fracrow[0:1, 512 * h:512 * (h + 1)],
                                 start=False, stop=True)
                tsb = bpool.tile([128, 512], F32, tag="tausb")
                nc.scalar.activation(tsb[:], psT[:], AF.Copy)
                tausb.append(tsb)

            # ---- stage B: masked softmax attention in [k, q] layout ---------
            psO = []
            for h in range(NH):
                psO_h = psbig.tile([65, 512], F32, tag="psO")
                psO.append(psO_h)

            for kt in range(QT):
                for h in range(NH):
                    psV = ps512.tile([128, 512], F32, tag="ps512")
                    nc.tensor.matmul(psV[:], kbT[:, 128 * kt:128 * (kt + 1)],
                                     qbT[:, 512 * h:512 * (h + 1)],
                                     start=True, stop=False)
                    nc.tensor.matmul(psV[:], wrow[0:1, 128 * kt:128 * (kt + 1)],
                                     onesrow[:], start=False, stop=True)
                    psP = ps512.tile([128, 512], F32, tag="ps512")
                    nc.tensor.matmul(psP[:],
                                     kT[:, 128 * kt:128 * (kt + 1)],
                                     qT[:, 512 * h:512 * (h + 1)],
                                     start=True, stop=True)
                    g16 = bpool.tile([128, 512], F16, tag="g16")
                    nc.vector.tensor_tensor(g16[:], psV[:], tausb[h][:],
                                            OP.is_ge)
                    e16 = bpool.tile([128, 512], F16, tag="e16")
                    nc.scalar.activation(e16[:], psP[:], AF.Exp, scale=0.125)
                    p16 = bpool.tile([128, 512], F16, tag="p16")
                    peng = nc.gpsimd if USE_POOL else nc.vector
                    peng.tensor_tensor(p16[:], e16[:], g16[:], OP.mult)
                    nc.tensor.matmul(psO[h][:], vA[:, kt, :], p16[:],
                                     start=(kt == 0), stop=(kt == QT - 1))

            # ---- normalize + transpose back + store -------------------------
            osb = opool.tile([64, S], F16, tag="osb")
            sgrow = rowpool.tile([1, S], F32, tag="sgrow")
            for h in range(NH):
                nc.scalar.activation(osb[:, 512 * h:512 * (h + 1)],
                                     psO[h][0:64, :], AF.Copy)
                nc.scalar.activation(sgrow[0:1, 512 * h:512 * (h + 1)],
                                     psO[h][64:65, :], AF.Copy)
            sgcol = stpool.tile([128, QT], F32, tag="sgcol")
            sgdr = drpool.tile([S], F32, tag="sgdr")
            nc.sync.dma_start(sgdr[:], sgrow[0:1, :])   # dram linear q-order
            nc.sync.dma_start(sgcol[:],
                              sgdr[:].rearrange("(t p) -> p t", p=128))
            rsg = stpool.tile([128, QT], F32, tag="rsg")
            nc.vector.reciprocal(rsg[:], sgcol[:])

            ofin = opool.tile([128, QT, D], F32, tag="ofin")
            for t in range(QT):
                psB = pssmall.tile([128, 64], F16, tag="pssm")
                nc.tensor.transpose(psB[:], osb[:, 128 * t:128 * (t + 1)],
                                    ident16[0:64, 0:64])
                nc.scalar.activation(ofin[:, t, :], psB[:], AF.Copy,
                                     scale=rsg[:, t:t + 1])
            nc.sync.dma_start(outd[p].rearrange("(t p) d -> p t d", p=128),
                              ofin[:])

    nc.compile()
    return nc


_NC = None


def _get_nc():
    global _NC
    if _NC is None:
        _NC = build_program()
    return _NC


def kernel(q, k, v, mask):
    q = np.ascontiguousarray(np.asarray(q, dtype=np.float32))
    k = np.ascontiguousarray(np.asarray(k, dtype=np.float32))
    v = np.ascontiguousarray(np.asarray(v, dtype=np.float32))
    # mask is all-zeros per the problem spec (fill: zeros); the kernel bakes
    # that in (softmax over selected keys is unaffected by adding zeros).
    assert np.all(np.asarray(mask) == 0.0), "kernel assumes zero mask"

    qf = q.reshape(B * H, S, D)
    kf = k.reshape(B * H, S, D)
    vf = v.reshape(B * H, S, D)
    in_maps = make_in_maps(qf, kf, vf)

    nc = _get_nc()
    res = run_bass_kernel_spmd(nc, in_maps, core_ids=list(range(NCORES)))
    outs = [res.results[c]["out"] for c in range(NCORES)]
    out = np.concatenate(outs, axis=0).reshape(B, H, S, D)
    return out.astype(np.float32)


# revision 10
# speedup vs baseline: 1.8140x; 1.2682x over previous
"""Binary-approximate sparse attention on 8 Trainium2 NeuronCores.

Reference semantics (per batch b, head h, query q):
  s      = sign(q) . sign(k)            -- integer scores in [-64, 64], even
  top-k  = 102 largest s, ties broken toward LOWER key index (jax.lax.top_k)
  out    = softmax over the precise scores (q.k/8) of the selected keys @ v

v3: multi-engine + software-pipelined across the 3 (b,h) pairs per core.
  - per-pair phases prep (loads/transposes/approx-score matmuls), phase1
    (threshold bisection + tie cutoff), stageb (masked softmax attention)
    are emitted interleaved: prep0 prep1 p1(0) prep2 p1(1) sb(0) p1(2)
    sb(1) sb(2) -- so each engine's in-order queue always has runnable
    work from another pair while one pair sits in a serial phase.
  - 6-step bisection over the odd candidate lattice T(i) = 2i - 65,
    window [0, 64]; count passes split DVE (fused is_ge+accum) / ACT
    (Sign activation with per-partition bias + free-dim accumulator:
    cnt = (acc + S)/2).
  - phase 2 is ONE custom-DVE pass per tile (TIE_CUT_ANT): a fused
    cumsum(s == tlev) < r comparison with accumulate gives the tie
    cutoff index directly.
  - stage B masks without gathers: augmented score s + (S-1-k)/S (wrow
    matmul into PSUM) compared against the per-query threshold tau
    broadcast via ones-matmuls; p = exp * mask multiply on gpsimd.
"""

import numpy as np

from contextlib import ExitStack

import concourse.bacc as bacc
import concourse.bass as bass
import concourse.mybir as mybir
import concourse.tile as tile
from concourse.bass_utils import run_bass_kernel_spmd

B, H, S, D = 2, 12, 1024, 64
NCORES = 8
PAIRS = (B * H) // NCORES          # (b,h) pairs per core
KP = 102                           # top-k
QT = S // 128                      # 128-row tiles per axis
NH = S // 512                      # 512-col halves
NITER = 6                          # bisection steps over window [0, 64]
ACT_TILES = 4                      # count tiles 0..3 on ACT, 4..7 on DVE
CAST_ACT = 4                       # sa16 cast tiles 0..3 on ACT, rest DVE
USE_POOL = True                    # p = e*g multiply on the gpsimd engine

F32 = mybir.dt.float32
F32R = mybir.dt.float32r
F16 = mybir.dt.float16
AF = mybir.ActivationFunctionType
OP = mybir.AluOpType


def _register_tie_cut():
    """Custom DVE op fusing phase 2 into one pass per tile:
      pre = cumsum(s == tlev); out = (pre < r); accum = #(pre < r) = c,
    the 0-based key index of the r-th tie (ties broken toward lower index).
    Replaces eq (tensor_scalar) + prefix scan + le-count, 3 full passes."""
    import concourse.dve_ops as dve_ops
    from concourse.dve_spec import Spec, Src0, C0, C1, AluOp, eq, scan, lower
    from concourse.dve_uop import DveOpSpec

    name = "TIE_CUT_ANT"
    if any(o.name == name for o in dve_ops.OPS):
        return next(o for o in dve_ops.OPS if o.name == name)

    def _ref(in0, in1, c0, c1, c2):
        pre = np.cumsum(in0.astype(np.float32) == c0, axis=1)
        out = (pre < c1).astype(np.float32)
        return out, out.sum(axis=1, keepdims=True)

    spec = Spec(body=scan(AluOp.ADD, eq(Src0, C0)) < C1, reference=_ref,
                accum=AluOp.ADD)
    row = dve_ops._CUSTOM_DVE_ROW_BASE + len(dve_ops.OPS)
    assert row < 0x20
    uops = lower(spec, ver="v3")
    sha3 = DveOpSpec(name=name, opcode=row, uops=uops,
                     rd1_en=dve_ops.has_src1(spec)).sha("v3")
    op = dve_ops.DveOp(name, spec, subdim=False, uops_sha={"v3": sha3})
    dve_ops.OPS.append(op)
    dve_ops._SUB_OPCODE_FOR_NAME[name] = row
    dve_ops.CUSTOM_DVE_SPECS[name] = spec
    return op


def _consts():
    ident32 = np.eye(128, dtype=np.float32)
    ident16 = np.eye(128, dtype=np.float16)
    onesrow = np.ones((1, 512), dtype=np.float16)
    ones1x128 = np.ones((1, 128), dtype=np.float16)
    wrow = (((S - 1) - np.arange(S, dtype=np.float32)) / S).astype(np.float16)[None, :]
    return ident32, ident16, onesrow, ones1x128, wrow


def make_in_maps(qf, kf, vf):
    ident32, ident16, onesrow, ones1x128, wrow = _consts()
    in_maps = []
    for c in range(NCORES):
        sl = slice(c * PAIRS, (c + 1) * PAIRS)
        in_maps.append({
            "q_in": qf[sl], "k_in": kf[sl], "v_in": vf[sl],
            "ident32": ident32, "ident16": ident16,
            "onesrow": onesrow, "ones1x128": ones1x128, "wrow": wrow,
        })
    return in_maps


def build_program():
    TIE_CUT = _register_tie_cut()
    nc = bacc.Bacc("TRN2", target_bir_lowering=False, debug=False,
                   num_devices=NCORES)

    qd = nc.dram_tensor("q_in", (PAIRS, S, D), F32, kind="ExternalInput").ap()
    kd = nc.dram_tensor("k_in", (PAIRS, S, D), F32, kind="ExternalInput").ap()
    vd = nc.dram_tensor("v_in", (PAIRS, S, D), F32, kind="ExternalInput").ap()
    identd = nc.dram_tensor("ident32", (128, 128), F32, kind="ExternalInput").ap()
    ident16d = nc.dram_tensor("ident16", (128, 128), F16, kind="ExternalInput").ap()
    onesrowd = nc.dram_tensor("onesrow", (1, 512), F16, kind="ExternalInput").ap()
    ones1x128d = nc.dram_tensor("ones1x128", (1, 128), F16, kind="ExternalInput").ap()
    wrowd = nc.dram_tensor("wrow", (1, S), F16, kind="ExternalInput").ap()
    outd = nc.dram_tensor("out", (PAIRS, S, D), F32, kind="ExternalOutput").ap()

    with tile.TileContext(nc) as tc, ExitStack() as ctx:
        cpool = ctx.enter_context(tc.tile_pool(name="consts", bufs=1))
        ident = cpool.tile([128, 128], F32)
        ident16 = cpool.tile([128, 128], F16)
        onesrow = cpool.tile([1, 512], F16)
        ones1x128 = cpool.tile([1, 128], F16)
        wrow = cpool.tile([1, S], F16)
        nc.sync.dma_start(ident[:], identd)
        nc.sync.dma_start(ident16[:], ident16d)
        nc.sync.dma_start(onesrow[:], onesrowd)
        nc.sync.dma_start(ones1x128[:], ones1x128d)
        nc.sync.dma_start(wrow[:], wrowd)

        inpool = ctx.enter_context(tc.tile_pool(name="inp", bufs=3))
        tpool = ctx.enter_context(tc.tile_pool(name="tposed", bufs=3))
        sapool = ctx.enter_context(tc.tile_pool(name="sa", bufs=3))
        stpool = ctx.enter_context(tc.tile_pool(name="state", bufs=3))
        jpool = ctx.enter_context(tc.tile_pool(name="junk", bufs=3))
        rowpool = ctx.enter_context(tc.tile_pool(name="rows", bufs=3))
        bpool = ctx.enter_context(tc.tile_pool(name="stageb", bufs=4))
        opool = ctx.enter_context(tc.tile_pool(name="outs", bufs=3))
        drpool = ctx.enter_context(tc.tile_pool(name="drscratch", bufs=3, space="DRAM"))
        pssmall = ctx.enter_context(tc.tile_pool(name="pssmall", bufs=2, space="PSUM"))
        ps512 = ctx.enter_context(tc.tile_pool(name="ps512", bufs=4, space="PSUM"))
        psbig = ctx.enter_context(tc.tile_pool(name="psbig", bufs=2, space="PSUM"))

        st = [dict() for _ in range(PAIRS)]

        def prep(p):
            s = st[p]
            qN = inpool.tile([128, QT, D], F32, tag="qN")
            kN = inpool.tile([128, QT, D], F32, tag="kN")
            vN = inpool.tile([128, QT, D], F32, tag="vN")
            nc.sync.dma_start(qN[:], qd[p].rearrange("(t p) d -> p t d", p=128))
            nc.sync.dma_start(kN[:], kd[p].rearrange("(t p) d -> p t d", p=128))
            nc.sync.dma_start(vN[:], vd[p].rearrange("(t p) d -> p t d", p=128))

            # v in f16 with a ones column appended (row 64 of p@V psum = sigma)
            vA = inpool.tile([128, QT, D + 1], F16, tag="vA")
            nc.scalar.copy(vA[:, :, 0:D], vN[:])
            nc.vector.memset(vA[:, :, D:D + 1], 1.0)
            s["vA"] = vA

            # transpose q,k to [d, s]
            qT = tpool.tile([64, S], F32R, tag="qT")
            kT = tpool.tile([64, S], F32R, tag="kT")
            for t in range(QT):
                pst = pssmall.tile([64, 128], F32, tag="pssm")
                nc.tensor.transpose(pst[:], qN[:, t, :], ident[:])
                nc.scalar.activation(qT[:, 128 * t:128 * (t + 1)], pst[:], AF.Copy)
                pst2 = pssmall.tile([64, 128], F32, tag="pssm")
                nc.tensor.transpose(pst2[:], kN[:, t, :], ident[:])
                nc.scalar.activation(kT[:, 128 * t:128 * (t + 1)], pst2[:], AF.Copy)
            s["qT"], s["kT"] = qT, kT

            qbT = tpool.tile([64, S], F16, tag="qbT")
            kbT = tpool.tile([64, S], F16, tag="kbT")
            nc.scalar.activation(qbT[:], qT[:], AF.Sign)
            nc.scalar.activation(kbT[:], kT[:], AF.Sign)
            s["qbT"], s["kbT"] = qbT, kbT

            # layout-A approx scores s[q, k] (f16, exact integers)
            sa16 = sapool.tile([128, QT, S], F16, tag="sa16")
            for t in range(QT):
                for h in range(NH):
                    psA = ps512.tile([128, 512], F32, tag="ps512")
                    nc.tensor.matmul(psA[:], qbT[:, 128 * t:128 * (t + 1)],
                                     kbT[:, 512 * h:512 * (h + 1)],
                                     start=True, stop=True)
                    dst = sa16[:, t, 512 * h:512 * (h + 1)]
                    if t < CAST_ACT:
                        nc.scalar.activation(dst, psA[:], AF.Copy)
                    else:
                        nc.vector.tensor_copy(dst, psA[:])
            s["sa16"] = sa16

        def phase1(p):
            s = st[p]
            sa16 = s["sa16"]
            # bisection over candidates T(i) = 2i - 65 (odd), i in [0, 64];
            # lo feasible, hi infeasible; pow-2 widths keep midpoints integral
            lo = stpool.tile([128, QT], F32, tag="lo")
            hi = stpool.tile([128, QT], F32, tag="hi")
            cnt_hi = stpool.tile([128, QT], F32, tag="cnth")
            nc.vector.memset(lo[:], 0.0)
            nc.vector.memset(hi[:], 64.0)
            nc.vector.memset(cnt_hi[:], 0.0)

            for it in range(NITER):
                # T(mid) = 2*(lo+hi)/2 - 65 = lo + hi - 65
                tq = stpool.tile([128, QT], F32, tag="tq")
                nc.vector.scalar_tensor_tensor(tq[:], lo[:], -65.0, hi[:],
                                               OP.add, OP.add)
                ntq = stpool.tile([128, QT], F32, tag="ntq")
                nc.vector.tensor_scalar(ntq[:], tq[:], -1.0, None, OP.mult)
                cnt = stpool.tile([128, QT], F32, tag="cnt")
                acc = stpool.tile([128, QT], F32, tag="acc")
                for t in range(QT):
                    if t < ACT_TILES:
                        ja = jpool.tile([128, S], F16, tag="ja")
                        nc.scalar.activation(ja[:], sa16[:, t, :], AF.Sign,
                                             bias=ntq[:, t:t + 1],
                                             accum_out=acc[:, t:t + 1])
                    else:
                        jd = jpool.tile([128, S], F16, tag="jd")
                        nc.vector.tensor_scalar(jd[:], sa16[:, t, :],
                                                tq[:, t:t + 1], None, OP.is_ge,
                                                OP.add,
                                                accum_out=cnt[:, t:t + 1])
                # ACT columns: cnt = acc/2 + S/2
                nc.vector.tensor_scalar(cnt[:, 0:ACT_TILES],
                                        acc[:, 0:ACT_TILES], 0.5, S / 2.0,
                                        OP.mult, OP.add)
                feas = stpool.tile([128, QT], mybir.dt.int32, tag="feas")
                nc.vector.tensor_scalar(feas[:], cnt[:], float(KP), None,
                                        OP.is_ge)
                mid = stpool.tile([128, QT], F32, tag="mid")
                nc.vector.tensor_scalar(mid[:], tq[:], 65.0, 0.5, OP.add,
                                        OP.mult)
                lo2 = stpool.tile([128, QT], F32, tag="lo")
                hi2 = stpool.tile([128, QT], F32, tag="hi")
                ch2 = stpool.tile([128, QT], F32, tag="cnth")
                nc.vector.select(lo2[:], feas[:], mid[:], lo[:])
                nc.vector.select(hi2[:], feas[:], hi[:], mid[:])
                nc.vector.select(ch2[:], feas[:], cnt_hi[:], cnt[:])
                lo, hi, cnt_hi = lo2, hi2, ch2

            # t_level = T(lo)+1 = 2*lo - 64 (even);  m = cnt_hi = #(s > t)
            tlev = stpool.tile([128, QT], F32, tag="tlev")
            nc.vector.tensor_scalar(tlev[:], lo[:], 2.0, -64.0, OP.mult, OP.add)
            # r = KP - cnt_hi  (rank of the last tie to keep)
            rq = stpool.tile([128, QT], F32, tag="rq")
            nc.vector.tensor_scalar(rq[:], cnt_hi[:], -1.0, float(KP),
                                    OP.mult, OP.add)

            # phase 2: tie cutoff index c_q -- one fused custom-DVE pass/tile
            ccnt = stpool.tile([128, QT], F32, tag="ccnt")
            for t in range(QT):
                jt = jpool.tile([128, S], F16, tag="jd")
                nc.vector._custom_dve(TIE_CUT, out=jt[:], in0=sa16[:, t, :],
                                      s0=tlev[:, t:t + 1], s1=rq[:, t:t + 1],
                                      accum_out=ccnt[:, t:t + 1])

            # tau components: t (f16-exact int) and (S-1-c)/S (f16-exact)
            t16 = stpool.tile([128, QT], F16, tag="t16")
            nc.vector.tensor_copy(t16[:], tlev[:])
            frac16 = stpool.tile([128, QT], F16, tag="frac16")
            nc.vector.tensor_scalar(frac16[:], ccnt[:], -1.0 / S,
                                    (S - 1.0) / S, OP.mult, OP.add)

            # flatten per-query columns to rows [1, S] (order q = 128*t + p)
            # via a DRAM bounce: SBUF partition-crossing DMAs don't balance.
            trow = rowpool.tile([1, S], F16, tag="trow")
            fracrow = rowpool.tile([1, S], F16, tag="fracrow")
            tdr = drpool.tile([S], F16, tag="tdr")
            fdr = drpool.tile([S], F16, tag="fdr")
            nc.sync.dma_start(tdr[:], t16[:])      # dram linear 8p + t
            nc.sync.dma_start(fdr[:], frac16[:])
            nc.sync.dma_start(trow[0:1, :],
                              tdr[:].rearrange("(p t) -> t p", p=128))
            nc.sync.dma_start(fracrow[0:1, :],
                              fdr[:].rearrange("(p t) -> t p", p=128))
            s["trow"], s["fracrow"] = trow, fracrow

        def stageb(p):
            s = st[p]
            qT, kT = s["qT"], s["kT"]
            qbT, kbT = s["qbT"], s["kbT"]
            vA = s["vA"]
            trow, fracrow = s["trow"], s["fracrow"]

            # tau = tlev_q + frac_q, replicated across partitions [128, 512]
            tausb = []
            for h in range(NH):
                psT = ps512.tile([128, 512], F32, tag="ps512")
                nc.tensor.matmul(psT[:], ones1x128[:],
                                 trow[0:1, 512 * h:512 * (h + 1)],
                                 start=True, stop=False)
                nc.tensor.matmul(psT[:], ones1x128[:],
                                 fracrow[0:1, 512 * h:512 * (h + 1)],
                                 start=False, stop=True)
                tsb = bpool.tile([128, 512], F32, tag="tausb")
                nc.scalar.activation(tsb[:], psT[:], AF.Copy)
                tausb.append(tsb)

            # masked softmax attention in [k, q] layout
            psO = []
            for h in range(NH):
                psO_h = psbig.tile([65, 512], F32, tag="psO")
                psO.append(psO_h)

            for kt in range(QT):
                for h in range(NH):
                    psV = ps512.tile([128, 512], F32, tag="ps512")
                    nc.tensor.matmul(psV[:], kbT[:, 128 * kt:128 * (kt + 1)],
                                     qbT[:, 512 * h:512 * (h + 1)],
                                     start=True, stop=False)
                    nc.tensor.matmul(psV[:], wrow[0:1, 128 * kt:128 * (kt + 1)],
                                     onesrow[:], start=False, stop=True)
                    psP = ps512.tile([128, 512], F32, tag="ps512")
                    nc.tensor.matmul(psP[:],
                                     kT[:, 128 * kt:128 * (kt + 1)],
                                     qT[:, 512 * h:512 * (h + 1)],
                                     start=True, stop=True)
                    g16 = bpool.tile([128, 512], F16, tag="g16")
                    nc.vector.tensor_tensor(g16[:], psV[:], tausb[h][:],
                                            OP.is_ge)
                    e16 = bpool.tile([128, 512], F16, tag="e16")
                    nc.scalar.activation(e16[:], psP[:], AF.Exp, scale=0.125)
                    p16 = bpool.tile([128, 512], F16, tag="p16")
                    peng = nc.gpsimd if USE_POOL else nc.vector
                    peng.tensor_tensor(p16[:], e16[:], g16[:], OP.mult)
                    nc.tensor.matmul(psO[h][:], vA[:, kt, :], p16[:],
                                     start=(kt == 0), stop=(kt == QT - 1))

            # normalize + transpose back + store
            osb = opool.tile([64, S], F16, tag="osb")
            sgrow = rowpool.tile([1, S], F32, tag="sgrow")
            for h in range(NH):
                nc.scalar.activation(osb[:, 512 * h:512 * (h + 1)],
                                     psO[h][0:64, :], AF.Copy)
                nc.scalar.activation(sgrow[0:1, 512 * h:512 * (h + 1)],
                                     psO[h][64:65, :], AF.Copy)
            sgcol = stpool.tile([128, QT], F32, tag="sgcol")
            sgdr = drpool.tile([S], F32, tag="sgdr")
            nc.sync.dma_start(sgdr[:], sgrow[0:1, :])   # dram linear q-order
            nc.sync.dma_start(sgcol[:],
                              sgdr[:].rearrange("(t p) -> p t", p=128))
            rsg = stpool.tile([128, QT], F32, tag="rsg")
            nc.vector.reciprocal(rsg[:], sgcol[:])

            ofin = opool.tile([128, QT, D], F32, tag="ofin")
            for t in range(QT):
                psB = pssmall.tile([128, 64], F16, tag="pssm")
                nc.tensor.transpose(psB[:], osb[:, 128 * t:128 * (t + 1)],
                                    ident16[0:64, 0:64])
                nc.scalar.activation(ofin[:, t, :], psB[:], AF.Copy,
                                     scale=rsg[:, t:t + 1])
            nc.sync.dma_start(outd[p].rearrange("(t p) d -> p t d", p=128),
                              ofin[:])

        # software pipeline across the 3 pairs
        prep(0)
        prep(1)
        phase1(0)
        prep(2)
        phase1(1)
        stageb(0)
        phase1(2)
        stageb(1)
        stageb(2)

    nc.compile()
    return nc


_NC = None


def _get_nc():
    global _NC
    if _NC is None:
        _NC = build_program()
    return _NC


def kernel(q, k, v, mask):
    q = np.ascontiguousarray(np.asarray(q, dtype=np.float32))
    k = np.ascontiguousarray(np.asarray(k, dtype=np.float32))
    v = np.ascontiguousarray(np.asarray(v, dtype=np.float32))
    # mask is all-zeros per the problem spec (fill: zeros); the kernel bakes
    # that in (softmax over selected keys is unaffected by adding zeros).
    assert np.all(np.asarray(mask) == 0.0), "kernel assumes zero mask"

    qf = q.reshape(B * H, S, D)
    kf = k.reshape(B * H, S, D)
    vf = v.reshape(B * H, S, D)
    in_maps = make_in_maps(qf, kf, vf)

    nc = _get_nc()
    res = run_bass_kernel_spmd(nc, in_maps, core_ids=list(range(NCORES)))
    outs = [res.results[c]["out"] for c in range(NCORES)]
    out = np.concatenate(outs, axis=0).reshape(B, H, S, D)
    return out.astype(np.float32)


# revision 12
# speedup vs baseline: 1.8982x; 1.0464x over previous
"""Binary-approximate sparse attention on 8 Trainium2 NeuronCores.

Reference semantics (per batch b, head h, query q):
  s      = sign(q) . sign(k)            -- integer scores in [-64, 64], even
  top-k  = 102 largest s, ties broken toward LOWER key index (jax.lax.top_k)
  out    = softmax over the precise scores (q.k/8) of the selected keys @ v

v3: multi-engine + software-pipelined across the 3 (b,h) pairs per core.
  - per-pair phases prep (loads/transposes/approx-score matmuls), phase1
    (threshold bisection + tie cutoff), stageb (masked softmax attention)
    are emitted interleaved: prep0 prep1 p1(0) prep2 p1(1) sb(0) p1(2)
    sb(1) sb(2) -- so each engine's in-order queue always has runnable
    work from another pair while one pair sits in a serial phase.
  - 6-step bisection over the odd candidate lattice T(i) = 2i - 65,
    window [0, 64]; count passes split DVE (fused is_ge+accum) / ACT
    (Sign activation with per-partition bias + free-dim accumulator:
    cnt = (acc + S)/2).
  - phase 2 is ONE custom-DVE pass per tile (TIE_CUT_ANT): a fused
    cumsum(s == tlev) < r comparison with accumulate gives the tie
    cutoff index directly.
  - stage B masks without gathers: augmented score s + (S-1-k)/S (wrow
    matmul into PSUM) compared against the per-query threshold tau
    broadcast via ones-matmuls; p = exp * mask multiply on gpsimd.
"""

import numpy as np

from contextlib import ExitStack

import concourse.bacc as bacc
import concourse.bass as bass
import concourse.mybir as mybir
import concourse.tile as tile
from concourse.bass_utils import run_bass_kernel_spmd

B, H, S, D = 2, 12, 1024, 64
NCORES = 8
PAIRS = (B * H) // NCORES          # (b,h) pairs per core
KP = 102                           # top-k
QT = S // 128                      # 128-row tiles per axis
NH = S // 512                      # 512-col halves
NITER = 6                          # bisection steps over window [0, 64]
ACT_TILES = 4                      # count tiles 0..3 on ACT, 4..7 on DVE
CAST_ACT = 4                       # sa16 cast tiles 0..3 on ACT, rest DVE
USE_POOL = True                    # p = e*g multiply on the gpsimd engine

F32 = mybir.dt.float32
F32R = mybir.dt.float32r
F16 = mybir.dt.float16
AF = mybir.ActivationFunctionType
OP = mybir.AluOpType


def _register_tie_cut():
    """Custom DVE op fusing phase 2 into one pass per tile:
      pre = cumsum(s == tlev); out = (pre < r); accum = #(pre < r) = c,
    the 0-based key index of the r-th tie (ties broken toward lower index).
    Replaces eq (tensor_scalar) + prefix scan + le-count, 3 full passes."""
    import concourse.dve_ops as dve_ops
    from concourse.dve_spec import Spec, Src0, C0, C1, AluOp, eq, scan, lower
    from concourse.dve_uop import DveOpSpec

    name = "TIE_CUT_ANT"
    if any(o.name == name for o in dve_ops.OPS):
        return next(o for o in dve_ops.OPS if o.name == name)

    def _ref(in0, in1, c0, c1, c2):
        pre = np.cumsum(in0.astype(np.float32) == c0, axis=1)
        out = (pre < c1).astype(np.float32)
        return out, out.sum(axis=1, keepdims=True)

    spec = Spec(body=scan(AluOp.ADD, eq(Src0, C0)) < C1, reference=_ref,
                accum=AluOp.ADD)
    row = dve_ops._CUSTOM_DVE_ROW_BASE + len(dve_ops.OPS)
    assert row < 0x20
    uops = lower(spec, ver="v3")
    sha3 = DveOpSpec(name=name, opcode=row, uops=uops,
                     rd1_en=dve_ops.has_src1(spec)).sha("v3")
    op = dve_ops.DveOp(name, spec, subdim=False, uops_sha={"v3": sha3})
    dve_ops.OPS.append(op)
    dve_ops._SUB_OPCODE_FOR_NAME[name] = row
    dve_ops.CUSTOM_DVE_SPECS[name] = spec
    return op


def _consts():
    ident32 = np.eye(128, dtype=np.float32)
    ident16 = np.eye(128, dtype=np.float16)
    onesrow = np.ones((1, 512), dtype=np.float16)
    ones1x128 = np.ones((1, 128), dtype=np.float16)
    wrow = (((S - 1) - np.arange(S, dtype=np.float32)) / S).astype(np.float16)[None, :]
    return ident32, ident16, onesrow, ones1x128, wrow


def make_in_maps(qf, kf, vf):
    ident32, ident16, onesrow, ones1x128, wrow = _consts()
    in_maps = []
    for c in range(NCORES):
        sl = slice(c * PAIRS, (c + 1) * PAIRS)
        in_maps.append({
            "q_in": qf[sl], "k_in": kf[sl], "v_in": vf[sl],
            "ident32": ident32, "ident16": ident16,
            "onesrow": onesrow, "ones1x128": ones1x128, "wrow": wrow,
        })
    return in_maps


def build_program():
    TIE_CUT = _register_tie_cut()
    nc = bacc.Bacc("TRN2", target_bir_lowering=False, debug=False,
                   num_devices=NCORES)

    qd = nc.dram_tensor("q_in", (PAIRS, S, D), F32, kind="ExternalInput").ap()
    kd = nc.dram_tensor("k_in", (PAIRS, S, D), F32, kind="ExternalInput").ap()
    vd = nc.dram_tensor("v_in", (PAIRS, S, D), F32, kind="ExternalInput").ap()
    identd = nc.dram_tensor("ident32", (128, 128), F32, kind="ExternalInput").ap()
    ident16d = nc.dram_tensor("ident16", (128, 128), F16, kind="ExternalInput").ap()
    onesrowd = nc.dram_tensor("onesrow", (1, 512), F16, kind="ExternalInput").ap()
    ones1x128d = nc.dram_tensor("ones1x128", (1, 128), F16, kind="ExternalInput").ap()
    wrowd = nc.dram_tensor("wrow", (1, S), F16, kind="ExternalInput").ap()
    outd = nc.dram_tensor("out", (PAIRS, S, D), F32, kind="ExternalOutput").ap()

    with tile.TileContext(nc) as tc, ExitStack() as ctx:
        cpool = ctx.enter_context(tc.tile_pool(name="consts", bufs=1))
        ident = cpool.tile([128, 128], F32)
        ident16 = cpool.tile([128, 128], F16)
        onesrow = cpool.tile([1, 512], F16)
        ones1x128 = cpool.tile([1, 128], F16)
        wrow = cpool.tile([1, S], F16)
        nc.sync.dma_start(ident[:], identd)
        nc.sync.dma_start(ident16[:], ident16d)
        nc.sync.dma_start(onesrow[:], onesrowd)
        nc.sync.dma_start(ones1x128[:], ones1x128d)
        nc.sync.dma_start(wrow[:], wrowd)

        inpool = ctx.enter_context(tc.tile_pool(name="inp", bufs=3))
        tpool = ctx.enter_context(tc.tile_pool(name="tposed", bufs=3))
        sapool = ctx.enter_context(tc.tile_pool(name="sa", bufs=3))
        stpool = ctx.enter_context(tc.tile_pool(name="state", bufs=3))
        jpool = ctx.enter_context(tc.tile_pool(name="junk", bufs=3))
        rowpool = ctx.enter_context(tc.tile_pool(name="rows", bufs=3))
        bpool = ctx.enter_context(tc.tile_pool(name="stageb", bufs=4))
        opool = ctx.enter_context(tc.tile_pool(name="outs", bufs=3))
        drpool = ctx.enter_context(tc.tile_pool(name="drscratch", bufs=3, space="DRAM"))
        pssmall = ctx.enter_context(tc.tile_pool(name="pssmall", bufs=2, space="PSUM"))
        ps512 = ctx.enter_context(tc.tile_pool(name="ps512", bufs=4, space="PSUM"))
        psbig = ctx.enter_context(tc.tile_pool(name="psbig", bufs=2, space="PSUM"))

        st = [dict() for _ in range(PAIRS)]

        def prep(p):
            s = st[p]
            qN = inpool.tile([128, QT, D], F32, tag="qN")
            kN = inpool.tile([128, QT, D], F32, tag="kN")
            vN = inpool.tile([128, QT, D], F32, tag="vN")
            nc.sync.dma_start(qN[:], qd[p].rearrange("(t p) d -> p t d", p=128))
            nc.sync.dma_start(kN[:], kd[p].rearrange("(t p) d -> p t d", p=128))
            nc.sync.dma_start(vN[:], vd[p].rearrange("(t p) d -> p t d", p=128))

            # v in f16 with a ones column appended (row 64 of p@V psum = sigma)
            vA = inpool.tile([128, QT, D + 1], F16, tag="vA")
            nc.scalar.copy(vA[:, :, 0:D], vN[:])
            nc.vector.memset(vA[:, :, D:D + 1], 1.0)
            s["vA"] = vA

            # transpose q,k to [d, s]; two 128-col transposes share one
            # PSUM tile so each PSUM->SBUF copy covers 256 columns
            qT = tpool.tile([64, S], F32R, tag="qT")
            kT = tpool.tile([64, S], F32R, tag="kT")
            for dst, srcN in ((qT, qN), (kT, kN)):
                for t in range(0, QT, 2):
                    pst = pssmall.tile([64, 256], F32, tag="pssm")
                    nc.tensor.transpose(pst[:, 0:128], srcN[:, t, :], ident[:])
                    nc.tensor.transpose(pst[:, 128:256], srcN[:, t + 1, :],
                                        ident[:])
                    nc.scalar.activation(dst[:, 128 * t:128 * (t + 2)],
                                         pst[:], AF.Copy)
            s["qT"], s["kT"] = qT, kT

            # sign tiles augmented with a 65th contraction row:
            # kbA row 64 = w_k, qbA row 64 = 1  ->  one K=65 matmul
            # computes s + w_k in stage B (no separate wrow matmul).
            qbA = tpool.tile([65, S], F16, tag="qbT")
            kbA = tpool.tile([65, S], F16, tag="kbT")
            nc.scalar.activation(qbA[0:64, :], qT[:], AF.Sign)
            nc.scalar.activation(kbA[0:64, :], kT[:], AF.Sign)
            nc.vector.memset(qbA[64:65, :], 1.0)
            nc.scalar.copy(kbA[64:65, :], wrow[:])
            s["qbT"], s["kbT"] = qbA, kbA

            qbT, kbT = s["qbT"], s["kbT"]
            # layout-A approx scores s[q, k] (f16, exact integers)
            sa16 = sapool.tile([128, QT, S], F16, tag="sa16")
            for t in range(QT):
                for h in range(NH):
                    psA = ps512.tile([128, 512], F32, tag="ps512")
                    nc.tensor.matmul(psA[:],
                                     qbT[0:64, 128 * t:128 * (t + 1)],
                                     kbT[0:64, 512 * h:512 * (h + 1)],
                                     start=True, stop=True)
                    dst = sa16[:, t, 512 * h:512 * (h + 1)]
                    if t < CAST_ACT:
                        nc.scalar.activation(dst, psA[:], AF.Copy)
                    else:
                        nc.vector.tensor_copy(dst, psA[:])
            s["sa16"] = sa16

        def phase1(p):
            s = st[p]
            sa16 = s["sa16"]
            # bisection over candidates T(i) = 2i - 65 (odd), i in [0, 64];
            # lo feasible, hi infeasible; pow-2 widths keep midpoints integral
            lo = stpool.tile([128, QT], F32, tag="lo")
            hi = stpool.tile([128, QT], F32, tag="hi")
            cnt_hi = stpool.tile([128, QT], F32, tag="cnth")
            nc.vector.memset(lo[:], 0.0)
            nc.vector.memset(hi[:], 64.0)
            nc.vector.memset(cnt_hi[:], 0.0)

            for it in range(NITER):
                # T(mid) = 2*(lo+hi)/2 - 65 = lo + hi - 65
                tq = stpool.tile([128, QT], F32, tag="tq")
                nc.vector.scalar_tensor_tensor(tq[:], lo[:], -65.0, hi[:],
                                               OP.add, OP.add)
                ntq = stpool.tile([128, QT], F32, tag="ntq")
                nc.scalar.activation(ntq[:], tq[:], AF.Copy, scale=-1.0)
                cnt = stpool.tile([128, QT], F32, tag="cnt")
                acc = stpool.tile([128, QT], F32, tag="acc")
                for t in range(QT):
                    if t < ACT_TILES:
                        ja = jpool.tile([128, S], F16, tag="ja")
                        nc.scalar.activation(ja[:], sa16[:, t, :], AF.Sign,
                                             bias=ntq[:, t:t + 1],
                                             accum_out=acc[:, t:t + 1])
                    else:
                        jd = jpool.tile([128, S], F16, tag="jd")
                        nc.vector.tensor_scalar(jd[:], sa16[:, t, :],
                                                tq[:, t:t + 1], None, OP.is_ge,
                                                OP.add,
                                                accum_out=cnt[:, t:t + 1])
                # ACT columns: cnt = acc/2 + S/2
                nc.vector.tensor_scalar(cnt[:, 0:ACT_TILES],
                                        acc[:, 0:ACT_TILES], 0.5, S / 2.0,
                                        OP.mult, OP.add)
                feas = stpool.tile([128, QT], mybir.dt.int32, tag="feas")
                nc.vector.tensor_scalar(feas[:], cnt[:], float(KP), None,
                                        OP.is_ge)
                mid = stpool.tile([128, QT], F32, tag="mid")
                nc.vector.tensor_scalar(mid[:], tq[:], 65.0, 0.5, OP.add,
                                        OP.mult)
                lo2 = stpool.tile([128, QT], F32, tag="lo")
                hi2 = stpool.tile([128, QT], F32, tag="hi")
                ch2 = stpool.tile([128, QT], F32, tag="cnth")
                nc.vector.select(lo2[:], feas[:], mid[:], lo[:])
                nc.vector.select(hi2[:], feas[:], hi[:], mid[:])
                nc.vector.select(ch2[:], feas[:], cnt_hi[:], cnt[:])
                lo, hi, cnt_hi = lo2, hi2, ch2

            # t_level = T(lo)+1 = 2*lo - 64 (even);  m = cnt_hi = #(s > t)
            tlev = stpool.tile([128, QT], F32, tag="tlev")
            nc.vector.tensor_scalar(tlev[:], lo[:], 2.0, -64.0, OP.mult, OP.add)
            # r = KP - cnt_hi  (rank of the last tie to keep)
            rq = stpool.tile([128, QT], F32, tag="rq")
            nc.vector.tensor_scalar(rq[:], cnt_hi[:], -1.0, float(KP),
                                    OP.mult, OP.add)

            # phase 2: tie cutoff index c_q -- one fused custom-DVE pass/tile
            ccnt = stpool.tile([128, QT], F32, tag="ccnt")
            for t in range(QT):
                jt = jpool.tile([128, S], F16, tag="jd")
                nc.vector._custom_dve(TIE_CUT, out=jt[:], in0=sa16[:, t, :],
                                      s0=tlev[:, t:t + 1], s1=rq[:, t:t + 1],
                                      accum_out=ccnt[:, t:t + 1])

            # tau components: t (f16-exact int) and (S-1-c)/S (f16-exact)
            t16 = stpool.tile([128, QT], F16, tag="t16")
            nc.vector.tensor_copy(t16[:], tlev[:])
            frac16 = stpool.tile([128, QT], F16, tag="frac16")
            nc.vector.tensor_scalar(frac16[:], ccnt[:], -1.0 / S,
                                    (S - 1.0) / S, OP.mult, OP.add)

            # flatten per-query columns to rows [1, S] (order q = 128*t + p)
            # via a DRAM bounce: SBUF partition-crossing DMAs don't balance.
            trow = rowpool.tile([1, S], F16, tag="trow")
            fracrow = rowpool.tile([1, S], F16, tag="fracrow")
            tdr = drpool.tile([S], F16, tag="tdr")
            fdr = drpool.tile([S], F16, tag="fdr")
            nc.sync.dma_start(tdr[:], t16[:])      # dram linear 8p + t
            nc.sync.dma_start(fdr[:], frac16[:])
            nc.sync.dma_start(trow[0:1, :],
                              tdr[:].rearrange("(p t) -> t p", p=128))
            nc.sync.dma_start(fracrow[0:1, :],
                              fdr[:].rearrange("(p t) -> t p", p=128))
            s["trow"], s["fracrow"] = trow, fracrow

        def stageb(p):
            s = st[p]
            qT, kT = s["qT"], s["kT"]
            qbT, kbT = s["qbT"], s["kbT"]
            vA = s["vA"]
            trow, fracrow = s["trow"], s["fracrow"]

            # tau = tlev_q + frac_q, replicated across partitions [128, 512]
            tausb = []
            for h in range(NH):
                psT = ps512.tile([128, 512], F32, tag="ps512")
                nc.tensor.matmul(psT[:], ones1x128[:],
                                 trow[0:1, 512 * h:512 * (h + 1)],
                                 start=True, stop=False)
                nc.tensor.matmul(psT[:], ones1x128[:],
                                 fracrow[0:1, 512 * h:512 * (h + 1)],
                                 start=False, stop=True)
                tsb = bpool.tile([128, 512], F32, tag="tausb")
                nc.scalar.activation(tsb[:], psT[:], AF.Copy)
                tausb.append(tsb)

            # masked softmax attention in [k, q] layout
            psO = []
            for h in range(NH):
                psO_h = psbig.tile([65, 512], F32, tag="psO")
                psO.append(psO_h)

            for kt in range(QT):
                for h in range(NH):
                    psV = ps512.tile([128, 512], F32, tag="ps512")
                    nc.tensor.matmul(psV[:], kbT[:, 128 * kt:128 * (kt + 1)],
                                     qbT[:, 512 * h:512 * (h + 1)],
                                     start=True, stop=True)
                    psP = ps512.tile([128, 512], F32, tag="ps512")
                    nc.tensor.matmul(psP[:],
                                     kT[:, 128 * kt:128 * (kt + 1)],
                                     qT[:, 512 * h:512 * (h + 1)],
                                     start=True, stop=True)
                    g16 = bpool.tile([128, 512], F16, tag="g16")
                    nc.vector.tensor_tensor(g16[:], psV[:], tausb[h][:],
                                            OP.is_ge)
                    e16 = bpool.tile([128, 512], F16, tag="e16")
                    nc.scalar.activation(e16[:], psP[:], AF.Exp, scale=0.125)
                    p16 = bpool.tile([128, 512], F16, tag="p16")
                    peng = nc.gpsimd if USE_POOL else nc.vector
                    peng.tensor_tensor(p16[:], e16[:], g16[:], OP.mult)
                    nc.tensor.matmul(psO[h][:], vA[:, kt, :], p16[:],
                                     start=(kt == 0), stop=(kt == QT - 1))

            # normalize + transpose back + store
            osb = opool.tile([64, S], F16, tag="osb")
            sgrow = rowpool.tile([1, S], F32, tag="sgrow")
            for h in range(NH):
                nc.scalar.activation(osb[:, 512 * h:512 * (h + 1)],
                                     psO[h][0:64, :], AF.Copy)
                nc.scalar.activation(sgrow[0:1, 512 * h:512 * (h + 1)],
                                     psO[h][64:65, :], AF.Copy)
            sgcol = stpool.tile([128, QT], F32, tag="sgcol")
            sgdr = drpool.tile([S], F32, tag="sgdr")
            nc.sync.dma_start(sgdr[:], sgrow[0:1, :])   # dram linear q-order
            nc.sync.dma_start(sgcol[:],
                              sgdr[:].rearrange("(t p) -> p t", p=128))
            rsg = stpool.tile([128, QT], F32, tag="rsg")
            nc.vector.reciprocal(rsg[:], sgcol[:])

            ofin = opool.tile([128, QT, D], F32, tag="ofin")
            for t in range(QT):
                psB = pssmall.tile([128, 64], F16, tag="pssm")
                nc.tensor.transpose(psB[:], osb[:, 128 * t:128 * (t + 1)],
                                    ident16[0:64, 0:64])
                nc.scalar.activation(ofin[:, t, :], psB[:], AF.Copy,
                                     scale=rsg[:, t:t + 1])
            nc.sync.dma_start(outd[p].rearrange("(t p) d -> p t d", p=128),
                              ofin[:])

        # software pipeline across the 3 pairs
        prep(0)
        prep(1)
        phase1(0)
        prep(2)
        phase1(1)
        stageb(0)
        phase1(2)
        stageb(1)
        stageb(2)

    nc.compile()
    return nc


_NC = None


def _get_nc():
    global _NC
    if _NC is None:
        _NC = build_program()
    return _NC


def kernel(q, k, v, mask):
    q = np.ascontiguousarray(np.asarray(q, dtype=np.float32))
    k = np.ascontiguousarray(np.asarray(k, dtype=np.float32))
    v = np.ascontiguousarray(np.asarray(v, dtype=np.float32))
    # mask is all-zeros per the problem spec (fill: zeros); the kernel bakes
    # that in (softmax over selected keys is unaffected by adding zeros).
    assert np.all(np.asarray(mask) == 0.0), "kernel assumes zero mask"

    qf = q.reshape(B * H, S, D)
    kf = k.reshape(B * H, S, D)
    vf = v.reshape(B * H, S, D)
    in_maps = make_in_maps(qf, kf, vf)

    nc = _get_nc()
    res = run_bass_kernel_spmd(nc, in_maps, core_ids=list(range(NCORES)))
    outs = [res.results[c]["out"] for c in range(NCORES)]
    out = np.concatenate(outs, axis=0).reshape(B, H, S, D)
    return out.astype(np.float32)
